# revision 1
# baseline (speedup 1.0000x reference)
"""BrainAgeGAT Trainium2 kernel: 2-layer GATv2 + mean-pool + MLP on 8 NeuronCores.

Strategy (per sharding_hint: shard edges; 1D-shard nodes; all-reduce pool):
  - Edges (incl. self loops) are sorted by destination and sharded by
    destination-node range across the 8 cores, so each core owns the full
    softmax/scatter for its destination nodes.
  - Per-core destination nodes are packed into blocks of <=127 "slots"
    (slot 127 of each 128-row block is a garbage slot for padding edges).
  - Node transforms xl = x@Wl / xr = x@Wr are computed on each core for its
    own node shard; the xl table is AllGather'd so every core can gather any
    source row. Per edge, 512-byte bf16 rows are fetched with dma_gather
    (SWDGE gather, int16 indices; the 51200-row global table is split in two
    halves to stay within int16).
  - u = xl[src]+xr[dst] on DVE; logits = per-head tree-reduction of
    att * leaky_relu(u) (ACT Lrelu + DVE); softmax needs no max subtraction
    at these magnitudes. Since softmax weights sum to 1, we scatter
    exp(logit)*u instead of exp(logit)*xl and subtract xr once per
    destination node at the end.
  - The segment scatter-sum is a one-hot matmul accumulated in PSUM
    (one-hot built on DVE from an iota tile and the per-edge slot offset).
  - Mean-pool uses per-block one-hot graph-selector matmuls into a
    persistent PSUM accumulator, an 8-core AllReduce, and a tiny MLP.
"""

import math
import sys

sys.path.insert(0, "/opt/trn_rl_repo")

import ml_dtypes
import numpy as np

import concourse.bacc as bacc
import concourse.bass as bass
import concourse.mybir as mybir
import concourse.tile as tile
from concourse import library_config
from concourse.vector_clock import ScopedClock

BF16 = ml_dtypes.bfloat16

# ---------------------------------------------------------------------------
# Patches for walrus' one-sync-wait-per-instruction limit.
# ---------------------------------------------------------------------------


def _drain_and_barrier(self, tick_clock, wait_clock):
    nc = self.nc
    probe = nc.sync.nop(nofuse=True, hint="drain_wait_split")
    wait_clock.add_sem_waits(probe.ins, ScopedClock({None: tick_clock.global_clock}))
    si = probe.ins.sync_info
    waits = list(si.on_wait) if si and si.on_wait else []
    if len(waits) > 1:
        si.on_wait = waits[:1]
        for w in waits[1:]:
            extra = nc.sync.nop(nofuse=True, hint="drain_wait_split")
            extra.ins.sync_info = type(si)(on_wait=[w], on_update=[])
    nc.sync.drain()
    nc.all_engine_barrier()
    assert self.sems is not None
    popped = nc._tile_sem_poison_stack.pop()
    assert popped is self._sem_poison
    nc.clear_and_free_semaphores(list(self.sems.allocated().values()))
    nc.all_engine_barrier()


tile.TileContext._drain_and_barrier = _drain_and_barrier


def _split_waits(nc):
    """walrus codegen accepts one sync-wait command per instruction; Tile can
    emit several. Hoist extras onto preceding same-engine NoOps."""
    for bb in nc.main_func.blocks:
        out = []
        for ins in bb.instructions:
            si = ins.sync_info
            waits = list(si.on_wait) if si and si.on_wait else []
            if len(waits) > 1:
                for w in waits[:-1]:
                    nop = mybir.InstNoOp(
                        name=nc.get_next_instruction_name(), ins=[], outs=[]
                    )
                    nop.engine = ins.engine
                    nop.sync_info = mybir.SyncInfo(on_wait=[w], on_update=[])
                    nc.register_instruction(nop)
                    out.append(nop)
                si.on_wait = [waits[-1]]
            out.append(ins)
        bb.instructions = out


# ---------------------------------------------------------------------------
# Model dimensions (hardcoded per problem spec)
# ---------------------------------------------------------------------------
N = 50000
E = 800000
G = 128
H = 8
C = 32
HC = H * C  # 256
P = 128
NCORES = 8
SLOTS = 127  # real slots per block (slot 127 = garbage)
MAXI16 = 25600  # table-piece size for int16 gather indices


class Cfg:
    """tba/tbb: per-block tile counts for the two xl-table pieces (uniform
    across cores so the SPMD program is identical)."""

    def __init__(self, n_nodes, ncores, nblk, tba, tbb):
        self.n_nodes = n_nodes
        self.ncores = ncores
        self.nodes_pc = n_nodes // ncores
        self.nblk = nblk
        self.cap = nblk * P
        self.capext = ncores * self.cap
        self.tba = tba  # list[nblk]
        self.tbb = tbb  # list[nblk]
        self.tb = [a + b for a, b in zip(tba, tbb)]
        self.ttot = sum(self.tb)
        self.col0 = np.concatenate([[0], np.cumsum(self.tb)]).astype(int)
        self.npiece = 2 if self.capext > MAXI16 else 1
        if self.npiece == 1:
            assert all(b == 0 for b in tbb)


# ---------------------------------------------------------------------------
# Host-side preprocessing
# ---------------------------------------------------------------------------


def _f32(a):
    return np.ascontiguousarray(a, dtype=np.float32)


def _bf(a):
    return np.ascontiguousarray(np.asarray(a, dtype=np.float32).astype(BF16))


def _wrap_idx(ids):
    """Gather-index list -> [128, len/16] int16 in the SWDGE wrap layout
    (idx j read from [j % 16, j // 16], replicated over the 8 Q7 cores)."""
    ids = np.asarray(ids, np.int16)
    assert len(ids) % 16 == 0
    w = ids.reshape(-1, 16).T  # [16, s]
    return np.tile(w, (8, 1))  # [128, s]


def _plan_blocks(edge_index, n_nodes, ncores):
    """Sort/pad edges; return per-core edge structures + uniform tile counts."""
    npc = n_nodes // ncores
    nblk = (npc + SLOTS - 1) // SLOTS
    cap = nblk * P
    capext = ncores * cap
    npiece = 2 if capext > MAXI16 else 1

    src = np.concatenate([edge_index[0], np.arange(n_nodes)]).astype(np.int64)
    dst = np.concatenate([edge_index[1], np.arange(n_nodes)]).astype(np.int64)
    order = np.argsort(dst, kind="stable")
    src, dst = src[order], dst[order]

    sloc = src % npc
    srow = (src // npc) * cap + (sloc // SLOTS) * P + (sloc % SLOTS)

    percore = []
    na = np.zeros((ncores, nblk), int)
    nb_ = np.zeros((ncores, nblk), int)
    for c in range(ncores):
        lo = c * npc
        sel = (dst >= lo) & (dst < lo + npc)
        bsrow = srow[sel]
        loc = dst[sel] - lo
        blocks = []
        for b in range(nblk):
            es = (loc // SLOTS) == b
            rs = bsrow[es]
            slots = (loc[es] % SLOTS).astype(np.int64)
            piece = rs // MAXI16 if npiece == 2 else np.zeros_like(rs)
            a_i = np.where(piece == 0)[0]
            b_i = np.where(piece == 1)[0]
            blocks.append((rs, slots, a_i, b_i))
            na[c, b] = len(a_i)
            nb_[c, b] = len(b_i)
        percore.append(blocks)
    # uniform per-block tile counts (pad so each block always has >=1 dummy
    # slot in piece A, keeping every block's garbage slot written)
    tba = [int(math.ceil((na[:, b].max() + 1) / P)) for b in range(nblk)]
    tbb = [int(math.ceil(nb_[:, b].max() / P)) if npiece == 2 else 0
           for b in range(nblk)]
    return percore, nblk, tba, tbb


def _prep(x, edge_index, batch, u, weights, cfg: Cfg, percore):
    npc = cfg.nodes_pc
    att1 = weights["att1"]
    att2 = weights["att2"]

    def att_rep(att):
        return _bf(np.broadcast_to(att.reshape(-1), (P, HC)))

    idx_cols_a = [t * 8 for t in cfg.tba]
    idx_cols_b = [t * 8 for t in cfg.tbb]

    maps = []
    for c in range(cfg.ncores):
        m = {}
        lo = c * npc
        ixa = np.zeros((P, sum(idx_cols_a)), np.int16)
        ixb = np.zeros((P, max(sum(idx_cols_b), 1)), np.int16)
        ixr = np.zeros((P, cfg.ttot * 8), np.int16)
        doffT = np.full((P, cfg.ttot), 127.0, np.float32)
        ca = cb = cr = 0
        for b in range(cfg.nblk):
            rs, slots, a_i, b_i = percore[c][b]
            garb_ext = c * cfg.cap + b * P + 127
            garb_loc = b * P + 127
            na, nb_ = len(a_i), len(b_i)
            ea = cfg.tba[b] * P
            eb = cfg.tbb[b] * P
            # piece-A gather indices (relative to piece), padded w/ garbage row
            ia = np.full(ea, garb_ext % MAXI16, np.int64)
            ia[:na] = rs[a_i] % MAXI16
            assert (c * cfg.cap + b * P + 127) // MAXI16 == 0 or cfg.npiece == 2
            if garb_ext >= MAXI16:
                # garbage row lives in piece B; pad piece A with row 0 of A
                ia[na:] = 0
            ib = np.full(eb, 0, np.int64)
            ib[:nb_] = rs[b_i] % MAXI16
            ixa[:, ca : ca + cfg.tba[b] * 8] = _wrap_idx(ia)
            if eb:
                ixb[:, cb : cb + cfg.tbb[b] * 8] = _wrap_idx(ib)
            # xr gather follows the same edge order [A | Apad | B | Bpad]
            rrow = np.full(ea + eb, garb_loc, np.int64)
            rrow[:na] = b * P + slots[a_i]
            rrow[ea : ea + nb_] = b * P + slots[b_i]
            ixr[:, cr : cr + cfg.tb[b] * 8] = _wrap_idx(rrow)
            # slot offsets in tile-major layout (edge i -> [i%128, i//128])
            off = np.full(ea + eb, 127.0, np.float32)
            off[:na] = slots[a_i]
            off[ea : ea + nb_] = slots[b_i]
            doffT[:, cfg.col0[b] : cfg.col0[b + 1]] = off.reshape(cfg.tb[b], P).T
            ca += cfg.tba[b] * 8
            cb += cfg.tbb[b] * 8
            cr += cfg.tb[b] * 8
        m["ixa"] = ixa
        m["ixb"] = ixb
        m["ixr"] = ixr
        m["doffT"] = doffT

        xs = np.zeros((cfg.cap, x.shape[1]), np.float32)
        rows = (np.arange(npc) // SLOTS) * P + (np.arange(npc) % SLOTS)
        xs[rows] = x[lo : lo + npc]
        m["xT"] = _bf(xs.T)

        gsel = np.zeros((cfg.cap, G), np.float32)
        gsel[rows, np.asarray(batch[lo : lo + npc])] = 1.0
        m["gsel"] = _bf(gsel)
        maps.append(m)

    counts = np.bincount(np.asarray(batch), minlength=G).astype(np.float32)
    shared = {
        "Wl1": _bf(weights["Wl1"]),
        "Wr1": _bf(weights["Wr1"]),
        "Wl2": _bf(weights["Wl2"]),
        "Wr2": _bf(weights["Wr2"]),
        "att1r": att_rep(att1),
        "att2r": att_rep(att2),
        "b1r": _bf(np.broadcast_to(weights["b1"], (P, HC))),
        "b2r": _bf(np.broadcast_to(weights["b2"], (P, HC))),
        "iotab": _bf(np.broadcast_to(np.arange(P, dtype=np.float32), (P, P))),
        "ident": _bf(np.eye(P, dtype=np.float32)),
        "crecip": _f32((1.0 / np.maximum(counts, 1.0)).reshape(G, 1)),
        "Wlin1": _bf(weights["W_lin1"]),
        "blin1r": _f32(np.broadcast_to(weights["b_lin1"], (G, 64))),
        "Wout": _bf(weights["W_out"]),
        "boutr": _f32(np.full((G, 1), float(weights["b_out"][0]), np.float32)),
        "ub": _bf(u),
    }
    for m in maps:
        m.update(shared)
    return maps


# ---------------------------------------------------------------------------
# Device program
# ---------------------------------------------------------------------------


def _bcast_mid(ap, reps):
    return ap.unsqueeze(1).broadcast_to([ap.shape[0], reps, ap.shape[1]])


def _build(cfg: Cfg, in_dim=3):
    dt = mybir.dt
    bf = dt.bfloat16
    f32 = dt.float32
    nc = bacc.Bacc(None)
    groups = [list(range(cfg.ncores))]

    def prm(name, shape, dtype):
        return nc.declare_dram_parameter(name, list(shape), dtype, isOutput=False)

    xT = prm("xT", [in_dim, cfg.cap], bf)
    ixa = prm("ixa", [P, sum(t * 8 for t in cfg.tba)], dt.int16)
    ixb = prm("ixb", [P, max(sum(t * 8 for t in cfg.tbb), 1)], dt.int16)
    ixr = prm("ixr", [P, cfg.ttot * 8], dt.int16)
    doffT = prm("doffT", [P, cfg.ttot], f32)
    Wl1p = prm("Wl1", [in_dim, HC], bf)
    Wr1p = prm("Wr1", [in_dim, HC], bf)
    Wl2p = prm("Wl2", [HC, HC], bf)
    Wr2p = prm("Wr2", [HC, HC], bf)
    att1r = prm("att1r", [P, HC], bf)
    att2r = prm("att2r", [P, HC], bf)
    b1r = prm("b1r", [P, HC], bf)
    b2r = prm("b2r", [P, HC], bf)
    iotab = prm("iotab", [P, P], bf)
    identp = prm("ident", [P, P], bf)
    gselp = prm("gsel", [cfg.cap, G], bf)
    crecip = prm("crecip", [G, 1], f32)
    Wlin1 = prm("Wlin1", [HC, 64], bf)
    blin1r = prm("blin1r", [G, 64], f32)
    Woutp = prm("Wout", [64 + 3, 1], bf)
    boutr = prm("boutr", [G, 1], f32)
    ub = prm("ub", [G, 3], bf)
    out_g = nc.declare_dram_parameter("out_g", [G, 1], f32, isOutput=True)

    with tile.TileContext(nc) as tc:
        with (
            tc.tile_pool(name="const", bufs=1) as constp,
            tc.tile_pool(name="meta", bufs=3) as metap,
            tc.tile_pool(name="gbuf", bufs=2) as gbufp,
            tc.tile_pool(name="work", bufs=2) as workp,
            tc.tile_pool(name="small", bufs=3) as smallp,
            tc.tile_pool(name="opool", bufs=4) as opool,
            tc.tile_pool(name="psA", bufs=2, space="PSUM") as psA,
            tc.tile_pool(name="psB", bufs=2, space="PSUM") as psB,
            tc.tile_pool(name="psG", bufs=1, space="PSUM") as psG,
            tc.tile_pool(name="dram", bufs=1, space="DRAM") as dram,
        ):
            # ---- constants to SBUF ----
            def cload(p):
                t = constp.tile([p.shape[0], p.shape[1]], p.dtype, name=p.name + "_s")
                nc.sync.dma_start(out=t[:], in_=p[:])
                return t

            def cload_k(p):
                nk = (p.shape[0] + P - 1) // P
                out = []
                for kt in range(nk):
                    rows = slice(kt * P, min((kt + 1) * P, p.shape[0]))
                    t = constp.tile(
                        [rows.stop - rows.start, p.shape[1]], p.dtype,
                        name=f"{p.name}_s{kt}",
                    )
                    nc.sync.dma_start(out=t[:], in_=p[rows, :])
                    out.append(t)
                return out

            xT_s = cload(xT)
            Wl1_s = cload_k(Wl1p)
            Wr1_s = cload_k(Wr1p)
            Wl2_s = cload_k(Wl2p)
            Wr2_s = cload_k(Wr2p)
            att1r_s = cload(att1r)
            att2r_s = cload(att2r)
            b1r_s = cload(b1r)
            b2r_s = cload(b2r)
            iotab_s = cload(iotab)
            ident_s = cload(identp)
            crecip_s = cload(crecip)
            Wlin1_s = cload_k(Wlin1)
            blin1r_s = cload(blin1r)
            Wout_s = cload(Woutp)
            boutr_s = cload(boutr)
            ub_s = cload(ub)

            # ---- internal DRAM ----
            xl1_own = dram.tile([cfg.cap, HC], bf)
            xr1_tab = dram.tile([cfg.cap, HC], bf)
            xrb1 = dram.tile([cfg.cap, HC], bf)
            xl1_ext = dram.tile([cfg.capext, HC], bf, addr_space="Shared")
            h1d = dram.tile([cfg.cap, HC], bf)
            h1T = dram.tile([2, P, cfg.cap], bf)
            xl2_own = dram.tile([cfg.cap, HC], bf)
            xr2_tab = dram.tile([cfg.cap, HC], bf)
            xrb2 = dram.tile([cfg.cap, HC], bf)
            xl2_ext = dram.tile([cfg.capext, HC], bf, addr_space="Shared")
            gp_in = dram.tile([G, HC], f32)
            gp_out = dram.tile([G, HC], f32, addr_space="Shared")

            A_ = mybir.AluOpType
            AF = mybir.ActivationFunctionType

            # ================= node tables =================
            def node_tables(lhsT_tiles, Wl_s, Wr_s, br_s, xl_dst, xr_dst, xrb_dst):
                for b in range(cfg.nblk):
                    rows = slice(b * P, (b + 1) * P)
                    for W_s, tab, extra in ((Wl_s, xl_dst, None), (Wr_s, xr_dst, xrb_dst)):
                        ps = psA.tile([P, HC], f32, tag="a")
                        lts = lhsT_tiles(b)
                        assert len(lts) == len(W_s)
                        for i, lt in enumerate(lts):
                            nc.tensor.matmul(
                                ps[:], lhsT=lt, rhs=W_s[i][:],
                                start=(i == 0), stop=(i == len(lts) - 1),
                            )
                        ev = smallp.tile([P, HC], bf, tag="tabev")
                        nc.scalar.activation(out=ev[:], in_=ps[:], func=AF.Copy)
                        nc.sync.dma_start(out=tab[rows, :], in_=ev[:])
                        if extra is not None:
                            xb = smallp.tile([P, HC], bf, tag="tabxb")
                            nc.vector.tensor_tensor(
                                out=xb[:], in0=br_s[:], in1=ps[:], op=A_.subtract
                            )
                            nc.sync.dma_start(out=extra[rows, :], in_=xb[:])

            node_tables(
                lambda b: [xT_s[:, b * P : (b + 1) * P]],
                Wl1_s, Wr1_s, b1r_s, xl1_own, xr1_tab, xrb1,
            )
            nc.gpsimd.collective_compute(
                "AllGather", A_.bypass, replica_groups=groups,
                ins=[xl1_own.opt()], outs=[xl1_ext.opt()],
            )

            # ================= edge pipeline =================
            def edge_layer(xl_ext, xr_tab, xrb, attr_s, layer):
                gpool_ps = None
                if layer == 2:
                    gpool_ps = psG.tile([G, HC], f32, name=f"gpool_ps{layer}")
                ca = cb = 0
                for b in range(cfg.nblk):
                    tb = cfg.tb[b]
                    tba, tbb = cfg.tba[b], cfg.tbb[b]
                    c0 = cfg.col0[b]
                    doff_t = metap.tile([P, tb], f32, tag="doff")
                    nc.sync.dma_start(out=doff_t[:], in_=doffT[:, c0 : c0 + tb])
                    xrb_blk = metap.tile([P, HC], bf, tag="xrb")
                    nc.sync.dma_start(out=xrb_blk[:], in_=xrb[b * P : (b + 1) * P, :])
                    ixa_t = metap.tile([P, tba * 8], dt.int16, tag="ixa")
                    nc.sync.dma_start(out=ixa_t[:], in_=ixa[:, ca : ca + tba * 8])
                    ixr_t = metap.tile([P, tb * 8], dt.int16, tag="ixr")
                    nc.sync.dma_start(out=ixr_t[:], in_=ixr[:, c0 * 8 : (c0 + tb) * 8])

                    CH = 5  # tiles per gather call (640 rows, HW-proven)

                    def chunked_gather(dst, dst_t0, n_tiles, table, idxt, idx_c0):
                        for q0 in range(0, n_tiles, CH):
                            q1 = min(q0 + CH, n_tiles)
                            nc.gpsimd.dma_gather(
                                out_ap=dst[:, dst_t0 + q0 : dst_t0 + q1, :],
                                in_ap=table,
                                idxs_ap=idxt[:, idx_c0 + q0 * 8 : idx_c0 + q1 * 8],
                                num_idxs=(q1 - q0) * P, num_idxs_reg=(q1 - q0) * P,
                                elem_size=HC,
                            )

                    gxl = gbufp.tile([P, tb, HC], bf, tag="gxl")
                    chunked_gather(
                        gxl, 0, tba,
                        xl_ext[0:MAXI16, :] if cfg.npiece == 2 else xl_ext[:],
                        ixa_t, 0,
                    )
                    if tbb:
                        ixb_t = metap.tile([P, tbb * 8], dt.int16, tag="ixb")
                        nc.sync.dma_start(out=ixb_t[:], in_=ixb[:, cb : cb + tbb * 8])
                        chunked_gather(
                            gxl, tba, tbb, xl_ext[MAXI16 : cfg.capext, :], ixb_t, 0
                        )
                    gxr = gbufp.tile([P, tb, HC], bf, tag="gxr")
                    chunked_gather(gxr, 0, tb, xr_tab[:], ixr_t, 0)
                    ut = gbufp.tile([P, tb, HC], bf, tag="ut")
                    nc.vector.tensor_tensor(out=ut[:], in0=gxl[:], in1=gxr[:], op=A_.add)

                    ft = workp.tile([P, tb, HC], bf, tag="ft")
                    nc.scalar.activation(out=ft[:], in_=ut[:], func=AF.Prelu, alpha=0.2)
                    Pt = workp.tile([P, tb, HC], bf, tag="Pt")
                    nc.vector.tensor_tensor(
                        out=Pt[:], in0=ft[:], in1=_bcast_mid(attr_s[:], tb), op=A_.mult
                    )
                    v = Pt[:].rearrange("p t (h c) -> p (t h) c", h=H)
                    t1 = workp.tile([P, tb * H, 16], bf, tag="t1")
                    nc.vector.tensor_tensor(out=t1[:], in0=v[:, :, 0:16], in1=v[:, :, 16:32], op=A_.add)
                    t2 = workp.tile([P, tb * H, 8], bf, tag="t2")
                    nc.vector.tensor_tensor(out=t2[:], in0=t1[:, :, 0:8], in1=t1[:, :, 8:16], op=A_.add)
                    t3 = workp.tile([P, tb * H, 4], bf, tag="t3")
                    nc.vector.tensor_tensor(out=t3[:], in0=t2[:, :, 0:4], in1=t2[:, :, 4:8], op=A_.add)
                    t4 = workp.tile([P, tb * H, 2], bf, tag="t4")
                    nc.vector.tensor_tensor(out=t4[:], in0=t3[:, :, 0:2], in1=t3[:, :, 2:4], op=A_.add)
                    lg = workp.tile([P, tb * H], bf, tag="lg")
                    nc.vector.tensor_tensor(
                        out=lg[:].unsqueeze(2), in0=t4[:, :, 0:1], in1=t4[:, :, 1:2], op=A_.add
                    )
                    ext = workp.tile([P, tb * H, C], bf, tag="ext")
                    nc.scalar.activation(
                        out=ext[:], in_=lg[:].to_broadcast([P, tb * H, C]), func=AF.Exp
                    )
                    msg = workp.tile([P, tb, HC], bf, tag="msg")
                    nc.vector.tensor_tensor(
                        out=msg[:], in0=ut[:],
                        in1=ext[:].rearrange("p (t h) c -> p t (h c)", t=tb),
                        op=A_.mult,
                    )

                    acc = psB.tile([P, HC], f32, tag="b")
                    accd = psB.tile([P, H], f32, tag="d")
                    exv = ext[:].rearrange("p (t h) c -> p t h c", t=tb)
                    for t in range(tb):
                        Ot = opool.tile([P, P], bf, tag="O")
                        nc.vector.tensor_scalar(
                            out=Ot[:], in0=iotab_s[:], scalar1=doff_t[:, t : t + 1],
                            scalar2=None, op0=A_.is_equal,
                        )
                        nc.tensor.matmul(
                            acc[:], lhsT=Ot[:], rhs=msg[:, t, :],
                            start=(t == 0), stop=(t == tb - 1),
                        )
                        nc.tensor.matmul(
                            accd[:], lhsT=Ot[:], rhs=exv[:, t, :, 0],
                            start=(t == 0), stop=(t == tb - 1),
                        )

                    denom = smallp.tile([P, H], f32, tag="denom")
                    nc.vector.tensor_scalar(
                        out=denom[:], in0=accd[:], scalar1=1e-20, scalar2=None,
                        op0=A_.max,
                    )
                    rec = smallp.tile([P, H], f32, tag="rec")
                    nc.vector.reciprocal(out=rec[:], in_=denom[:])
                    hsc = smallp.tile([P, HC], bf, tag="hsc")
                    nc.vector.tensor_tensor(
                        out=hsc[:].rearrange("p (h c) -> p h c", h=H),
                        in0=acc[:].rearrange("p (h c) -> p h c", h=H),
                        in1=rec[:].to_broadcast([P, H, C]),
                        op=A_.mult,
                    )
                    hfin = smallp.tile([P, HC], bf, tag="hfin")
                    nc.vector.tensor_tensor(out=hfin[:], in0=hsc[:], in1=xrb_blk[:], op=A_.add)
                    hout = smallp.tile([P, HC], bf, tag="hout")
                    nc.scalar.activation(out=hout[:], in_=hfin[:], func=AF.Relu)

                    if layer == 1:
                        nc.sync.dma_start(out=h1d[b * P : (b + 1) * P, :], in_=hout[:])
                        for kt in range(2):
                            tp = psA.tile([P, P], bf, tag="a")
                            nc.tensor.transpose(
                                out=tp[:], in_=hout[:, kt * P : (kt + 1) * P],
                                identity=ident_s[:],
                            )
                            tps = smallp.tile([P, P], bf, tag="htps")
                            nc.scalar.activation(out=tps[:], in_=tp[:], func=AF.Copy)
                            nc.sync.dma_start(
                                out=h1T[kt, :, b * P : (b + 1) * P], in_=tps[:]
                            )
                    else:
                        gsel_blk = metap.tile([P, G], bf, tag="gselb")
                        nc.sync.dma_start(
                            out=gsel_blk[:], in_=gselp[b * P : (b + 1) * P, :]
                        )
                        nc.tensor.matmul(
                            gpool_ps[:], lhsT=gsel_blk[:], rhs=hout[:],
                            start=(b == 0), stop=(b == cfg.nblk - 1),
                        )
                    ca += tba * 8
                    cb += tbb * 8
                return gpool_ps

            edge_layer(xl1_ext, xr1_tab, xrb1, att1r_s, layer=1)

            # ================= layer-2 node tables =================
            def h1_lhsT(b):
                outs = []
                for kt in range(2):
                    t = smallp.tile([P, P], bf, tag="h1l", name=f"h1l{b}_{kt}")
                    nc.sync.dma_start(out=t[:], in_=h1T[kt, :, b * P : (b + 1) * P])
                    outs.append(t[:])
                return outs

            node_tables(h1_lhsT, Wl2_s, Wr2_s, b2r_s, xl2_own, xr2_tab, xrb2)
            nc.gpsimd.collective_compute(
                "AllGather", A_.bypass, replica_groups=groups,
                ins=[xl2_own.opt()], outs=[xl2_ext.opt()],
            )

            gpool_ps = edge_layer(xl2_ext, xr2_tab, xrb2, att2r_s, layer=2)

            # ================= pool + MLP =================
            A_ = mybir.AluOpType
            AF = mybir.ActivationFunctionType
            gsum = smallp.tile([G, HC], f32, tag="gsum")
            nc.scalar.activation(out=gsum[:], in_=gpool_ps[:], func=AF.Copy)
            nc.sync.dma_start(out=gp_in[:], in_=gsum[:])
            nc.gpsimd.collective_compute(
                "AllReduce", A_.add, replica_groups=groups,
                ins=[gp_in.opt()], outs=[gp_out.opt()],
            )
            gsum2 = smallp.tile([G, HC], f32, tag="gsum2")
            nc.sync.dma_start(out=gsum2[:], in_=gp_out[:])
            gmean = smallp.tile([G, HC], bf, tag="gmean")
            nc.vector.tensor_scalar(
                out=gmean[:], in0=gsum2[:], scalar1=crecip_s[:, 0:1], scalar2=None,
                op0=A_.mult,
            )
            gT = []
            for kt in range(2):
                tp = psA.tile([P, G], bf, tag="a")
                nc.tensor.transpose(
                    out=tp[:], in_=gmean[:, kt * P : (kt + 1) * P], identity=ident_s[:]
                )
                gkt = smallp.tile([P, G], bf, tag="gT", name=f"gT{kt}")
                nc.scalar.activation(out=gkt[:], in_=tp[:], func=AF.Copy)
                gT.append(gkt)
            lin_ps = psB.tile([G, 64], f32, tag="b")
            for kt in range(2):
                nc.tensor.matmul(
                    lin_ps[:], lhsT=gT[kt][:], rhs=Wlin1_s[kt][:],
                    start=(kt == 0), stop=(kt == 1),
                )
            lin = smallp.tile([G, 64], f32, tag="lin")
            nc.vector.tensor_tensor(out=lin[:], in0=lin_ps[:], in1=blin1r_s[:], op=A_.add)
            glu = smallp.tile([G, P], bf, tag="glu")
            nc.scalar.activation(out=glu[:, 0:64], in_=lin[:], func=AF.Relu)
            nc.vector.tensor_copy(out=glu[:, 64:67], in_=ub_s[:])
            nc.gpsimd.memset(glu[:, 67:P], 0.0)
            tp = psA.tile([P, G], bf, tag="a")
            nc.tensor.transpose(out=tp[:], in_=glu[:], identity=ident_s[:])
            gluT = smallp.tile([P, G], bf, tag="gluT")
            nc.scalar.activation(out=gluT[:], in_=tp[:], func=AF.Copy)
            out_ps = psB.tile([G, 1], f32, tag="b")
            nc.tensor.matmul(
                out_ps[:], lhsT=gluT[0:67, :], rhs=Wout_s[:], start=True, stop=True
            )
            outs = smallp.tile([G, 1], f32, tag="outs")
            nc.vector.tensor_tensor(out=outs[:], in0=out_ps[:], in1=boutr_s[:], op=A_.add)
            nc.sync.dma_start(out=out_g[:], in_=outs[:])

    nc.compile()
    _split_waits(nc)
    return nc


# ---------------------------------------------------------------------------
# Entry point
# ---------------------------------------------------------------------------


def kernel(**inputs):
    import os

    from concourse.bass_utils import run_bass_kernel_spmd

    x = np.asarray(inputs["x"], np.float32)
    edge_index = np.asarray(inputs["edge_index"], np.int64)
    batch = np.asarray(inputs["batch"], np.int64)
    u = np.asarray(inputs["u"], np.float32)
    weights = {
        k: np.asarray(inputs[k], np.float32)
        for k in ("Wl1", "Wr1", "att1", "b1", "Wl2", "Wr2", "att2", "b2",
                  "W_lin1", "b_lin1", "W_out", "b_out")
    }
    percore, nblk, tba, tbb = _plan_blocks(edge_index, N, NCORES)
    cfg = Cfg(N, NCORES, nblk, tba, tbb)
    maps = _prep(x, edge_index, batch, u, weights, cfg, percore)
    nc = _build(cfg, in_dim=x.shape[1])
    trace = bool(os.environ.get("KERNEL_TRACE"))
    try:
        res = run_bass_kernel_spmd(nc, maps, list(range(NCORES)), trace=trace)
    except ModuleNotFoundError:
        res = run_bass_kernel_spmd(nc, maps, list(range(NCORES)))
    if trace and getattr(res, "exec_time_ns", None) is not None:
        print(f"HW exec time: {res.exec_time_ns} ns")
    return res.results[0]["out_g"].reshape(G).astype(np.float32)



# revision 7
# speedup vs baseline: 2.5649x; 2.5649x over previous
"""BrainAgeGAT Trainium2 kernel: 2-layer GATv2 + mean-pool + MLP on 8 NeuronCores.

Strategy (v2):
  - Edges (incl. self loops) sharded by destination across the 8 cores; within
    a core, destination nodes are LPT-packed into 50 blocks of <=127 slots
    (slot 127 = garbage) so per-block edge counts are balanced and every block
    uses the same uniform tile counts (tba piece-A tiles + tbb piece-B tiles).
  - xl = x@Wl is AllGather'd; per edge a 512-byte bf16 row is fetched with
    dma_gather (SWDGE, int16 indices; the 51200-row table is split in two
    25600-row halves to stay within int16).
  - xr[dst] is NOT gathered: per block the 128-row xr slice is SBUF-resident
    and expanded per edge with a one-hot matmul (lhsT = OtT streamed from
    host) into PSUM, then copied to SBUF by ACT. The same one-hot (untransposed
    Ot, also host-streamed) drives the scatter-sum matmuls.
  - u = xl[src]+xr[dst] on DVE; logits = per-head tree-reduction of
    att * leaky_relu(u) (ACT Prelu + DVE); softmax needs no max subtraction at
    these magnitudes. Since softmax weights sum to 1, we scatter exp(logit)*u
    and subtract xr once per destination at the end. exp runs on the tiny
    [P, tb*H, 2] logit pair (not the 32x broadcast).
  - Mean-pool via per-block one-hot graph-selector matmuls into a persistent
    PSUM accumulator, an 8-core AllReduce, and a tiny MLP.
"""

import math
import sys

sys.path.insert(0, "/opt/trn_rl_repo")

import ml_dtypes
import numpy as np

import concourse.bacc as bacc
import concourse.bass as bass
import concourse.mybir as mybir
import concourse.tile as tile
from concourse import library_config
from concourse.vector_clock import ScopedClock

BF16 = ml_dtypes.bfloat16

# ---------------------------------------------------------------------------
# Patches for walrus' one-sync-wait-per-instruction limit.
# ---------------------------------------------------------------------------


def _drain_and_barrier(self, tick_clock, wait_clock):
    nc = self.nc
    probe = nc.sync.nop(nofuse=True, hint="drain_wait_split")
    wait_clock.add_sem_waits(probe.ins, ScopedClock({None: tick_clock.global_clock}))
    si = probe.ins.sync_info
    waits = list(si.on_wait) if si and si.on_wait else []
    if len(waits) > 1:
        si.on_wait = waits[:1]
        for w in waits[1:]:
            extra = nc.sync.nop(nofuse=True, hint="drain_wait_split")
            extra.ins.sync_info = type(si)(on_wait=[w], on_update=[])
    nc.sync.drain()
    nc.all_engine_barrier()
    assert self.sems is not None
    popped = nc._tile_sem_poison_stack.pop()
    assert popped is self._sem_poison
    nc.clear_and_free_semaphores(list(self.sems.allocated().values()))
    nc.all_engine_barrier()


tile.TileContext._drain_and_barrier = _drain_and_barrier


def _split_waits(nc):
    """walrus codegen accepts one sync-wait command per instruction; Tile can
    emit several. Hoist extras onto preceding same-engine NoOps."""
    for bb in nc.main_func.blocks:
        out = []
        for ins in bb.instructions:
            si = ins.sync_info
            waits = list(si.on_wait) if si and si.on_wait else []
            if len(waits) > 1:
                for w in waits[:-1]:
                    nop = mybir.InstNoOp(
                        name=nc.get_next_instruction_name(), ins=[], outs=[]
                    )
                    nop.engine = ins.engine
                    nop.sync_info = mybir.SyncInfo(on_wait=[w], on_update=[])
                    nc.register_instruction(nop)
                    out.append(nop)
                si.on_wait = [waits[-1]]
            out.append(ins)
        bb.instructions = out


# ---------------------------------------------------------------------------
# Model dimensions (hardcoded per problem spec)
# ---------------------------------------------------------------------------
N = 50000
E = 800000
G = 128
H = 8
C = 32
HC = H * C  # 256
P = 128
NCORES = 8
SLOTS = 127  # real slots per block (slot 127 = garbage)
MAXI16 = 25600  # table-piece size for int16 gather indices
NPC = N // NCORES  # 6250
NBLK = (NPC + SLOTS - 1) // SLOTS  # 50
CAP = NBLK * P  # 6400
CAPEXT = NCORES * CAP  # 51200
CH = 5  # gather tiles per dma_gather call
NQ = 4  # SWDGE queues to rotate gathers over


class Cfg:
    def __init__(self, tba, tbb):
        self.tba = tba
        self.tbb = tbb
        self.tb = tba + tbb
        self.ttot = NBLK * self.tb


# ---------------------------------------------------------------------------
# Host-side preprocessing
# ---------------------------------------------------------------------------


def _f32(a):
    return np.ascontiguousarray(a, dtype=np.float32)


def _bf(a):
    return np.ascontiguousarray(np.asarray(a, dtype=np.float32).astype(BF16))


def _wrap_idx(ids):
    """Gather-index list -> [128, len/16] int16 in the SWDGE wrap layout
    (idx j read from [j % 16, j // 16], replicated over the 8 Q7 cores)."""
    ids = np.asarray(ids, np.int16)
    assert len(ids) % 16 == 0
    w = ids.reshape(-1, 16).T  # [16, s]
    return np.tile(w, (8, 1))  # [128, s]


def _plan_blocks(edge_index):
    """LPT-pack dst nodes into blocks; return assignment + per-core edge
    structures + uniform tile counts."""
    src = np.concatenate([edge_index[0], np.arange(N)]).astype(np.int64)
    dst = np.concatenate([edge_index[1], np.arange(N)]).astype(np.int64)
    pieceB = (src // NPC) >= (NCORES // 2)
    dega = np.bincount(dst[~pieceB], minlength=N)
    degb = np.bincount(dst[pieceB], minlength=N)

    blk_of = np.empty(N, np.int64)
    slot_of = np.empty(N, np.int64)
    for c in range(NCORES):
        lo = c * NPC
        da = dega[lo : lo + NPC].astype(np.float64)
        db = degb[lo : lo + NPC].astype(np.float64)
        order = np.argsort(-(da + db), kind="stable")
        blk_a = np.zeros(NBLK)
        blk_b = np.zeros(NBLK)
        blk_n = np.zeros(NBLK, np.int64)
        for i in order:
            cost = np.maximum(blk_a + da[i], blk_b + db[i])
            cost[blk_n >= SLOTS] = np.inf
            j = int(np.argmin(cost))
            blk_of[lo + i] = j
            slot_of[lo + i] = blk_n[j]
            blk_a[j] += da[i]
            blk_b[j] += db[i]
            blk_n[j] += 1
    row_of = blk_of * P + slot_of  # within-core table row
    ext_row = (np.arange(N) // NPC) * CAP + row_of  # global table row

    # per-(core, block, piece) edge lists
    esrow = ext_row[src]
    eslot = slot_of[dst]
    eblk = blk_of[dst]
    ecore = dst // NPC
    percore = []
    na = np.zeros((NCORES, NBLK), int)
    nb_ = np.zeros((NCORES, NBLK), int)
    for c in range(NCORES):
        blocks = []
        selc = ecore == c
        for b in range(NBLK):
            sel = selc & (eblk == b)
            sa = sel & ~pieceB
            sb = sel & pieceB
            ra, la = esrow[sa], eslot[sa]
            rb, lb = esrow[sb] - MAXI16, eslot[sb]
            blocks.append((ra, la, rb, lb))
            na[c, b] = len(ra)
            nb_[c, b] = len(rb)
        percore.append(blocks)
    tba = int(math.ceil(na.max() / P))
    tbb = int(math.ceil(nb_.max() / P))
    return percore, row_of, Cfg(tba, tbb)


def _prep(x, batch, u, weights, cfg: Cfg, percore, row_of):
    att1 = weights["att1"]
    att2 = weights["att2"]

    def att_rep(att):
        return _bf(np.broadcast_to(att.reshape(-1), (P, HC)))

    tba, tbb, tb = cfg.tba, cfg.tbb, cfg.tb
    iota = np.arange(P)

    maps = []
    for c in range(NCORES):
        m = {}
        lo = c * NPC
        ixa = np.zeros((P, NBLK * tba * 8), np.int16)
        ixb = np.zeros((P, NBLK * tbb * 8), np.int16)
        Ot = np.zeros((P, NBLK * tb * P), BF16)
        OtT = np.zeros((P, NBLK * tb * P), BF16)
        for b in range(NBLK):
            ra, la, rb, lb = percore[c][b]
            ia = np.zeros(tba * P, np.int64)
            ia[: len(ra)] = ra
            ib = np.zeros(tbb * P, np.int64)
            ib[: len(rb)] = rb
            ixa[:, b * tba * 8 : (b + 1) * tba * 8] = _wrap_idx(ia)
            ixb[:, b * tbb * 8 : (b + 1) * tbb * 8] = _wrap_idx(ib)
            slots = np.full(tb * P, 127, np.int64)
            slots[: len(la)] = la
            slots[tba * P : tba * P + len(lb)] = lb
            oh = (slots[:, None] == iota[None, :]).astype(BF16)  # [tb*P, P]
            oh = oh.reshape(tb, P, P)
            cols = slice(b * tb * P, (b + 1) * tb * P)
            Ot[:, cols] = oh.transpose(1, 0, 2).reshape(P, tb * P)
            OtT[:, cols] = oh.transpose(2, 0, 1).reshape(P, tb * P)
        m["ixa"] = ixa
        m["ixb"] = ixb
        m["Ot"] = np.ascontiguousarray(Ot)
        m["OtT"] = np.ascontiguousarray(OtT)

        rows = row_of[lo : lo + NPC]
        xs = np.zeros((CAP, x.shape[1]), np.float32)
        xs[rows] = x[lo : lo + NPC]
        m["xT"] = _bf(xs.T)

        gsel = np.zeros((CAP, G), np.float32)
        gsel[rows, np.asarray(batch[lo : lo + NPC])] = 1.0
        m["gsel"] = _bf(gsel)
        maps.append(m)

    counts = np.bincount(np.asarray(batch), minlength=G).astype(np.float32)
    shared = {
        "Wl1": _bf(weights["Wl1"]),
        "Wr1": _bf(weights["Wr1"]),
        "Wl2": _bf(weights["Wl2"]),
        "Wr2": _bf(weights["Wr2"]),
        "att1r": att_rep(att1),
        "att2r": att_rep(att2),
        "b1r": _bf(np.broadcast_to(weights["b1"], (P, HC))),
        "b2r": _bf(np.broadcast_to(weights["b2"], (P, HC))),
        "ident": _bf(np.eye(P, dtype=np.float32)),
        "crecip": _f32((1.0 / np.maximum(counts, 1.0)).reshape(G, 1)),
        "Wlin1": _bf(weights["W_lin1"]),
        "blin1r": _f32(np.broadcast_to(weights["b_lin1"], (G, 64))),
        "Wout": _bf(weights["W_out"]),
        "boutr": _f32(np.full((G, 1), float(weights["b_out"][0]), np.float32)),
        "ub": _bf(u),
    }
    for m in maps:
        m.update(shared)
    return maps


# ---------------------------------------------------------------------------
# Device program
# ---------------------------------------------------------------------------


def _bcast_mid(ap, reps):
    return ap.unsqueeze(1).broadcast_to([ap.shape[0], reps, ap.shape[1]])


def _build(cfg: Cfg, in_dim=3):
    dt = mybir.dt
    bf = dt.bfloat16
    f32 = dt.float32
    nc = bacc.Bacc(None, num_swdge_queues=NQ) if NQ > 1 else bacc.Bacc(None)
    groups = [list(range(NCORES))]
    tba, tbb, tb = cfg.tba, cfg.tbb, cfg.tb

    def prm(name, shape, dtype):
        return nc.declare_dram_parameter(name, list(shape), dtype, isOutput=False)

    xT = prm("xT", [in_dim, CAP], bf)
    ixa = prm("ixa", [P, NBLK * tba * 8], dt.int16)
    ixb = prm("ixb", [P, NBLK * tbb * 8], dt.int16)
    Otp = prm("Ot", [P, NBLK * tb * P], bf)
    OtTp = prm("OtT", [P, NBLK * tb * P], bf)
    Wl1p = prm("Wl1", [in_dim, HC], bf)
    Wr1p = prm("Wr1", [in_dim, HC], bf)
    Wl2p = prm("Wl2", [HC, HC], bf)
    Wr2p = prm("Wr2", [HC, HC], bf)
    att1r = prm("att1r", [P, HC], bf)
    att2r = prm("att2r", [P, HC], bf)
    b1r = prm("b1r", [P, HC], bf)
    b2r = prm("b2r", [P, HC], bf)
    identp = prm("ident", [P, P], bf)
    gselp = prm("gsel", [CAP, G], bf)
    crecip = prm("crecip", [G, 1], f32)
    Wlin1 = prm("Wlin1", [HC, 64], bf)
    blin1r = prm("blin1r", [G, 64], f32)
    Woutp = prm("Wout", [64 + 3, 1], bf)
    boutr = prm("boutr", [G, 1], f32)
    ub = prm("ub", [G, 3], bf)
    out_g = nc.declare_dram_parameter("out_g", [G, 1], f32, isOutput=True)

    with tile.TileContext(nc) as tc:
        with (
            tc.tile_pool(name="const", bufs=1) as constp,
            tc.tile_pool(name="meta", bufs=2) as metap,
            tc.tile_pool(name="gbuf", bufs=2) as gbufp,
            tc.tile_pool(name="work", bufs=2) as workp,
            tc.tile_pool(name="small", bufs=3) as smallp,
            tc.tile_pool(name="psU", bufs=2, space="PSUM") as psU,
            tc.tile_pool(name="psA", bufs=2, space="PSUM") as psA,
            tc.tile_pool(name="psB", bufs=2, space="PSUM") as psB,
            tc.tile_pool(name="psG", bufs=1, space="PSUM") as psG,
            tc.tile_pool(name="dram", bufs=1, space="DRAM") as dram,
        ):
            # ---- constants to SBUF ----
            def cload(p):
                t = constp.tile([p.shape[0], p.shape[1]], p.dtype, name=p.name + "_s")
                nc.sync.dma_start(out=t[:], in_=p[:])
                return t

            def cload_k(p):
                nk = (p.shape[0] + P - 1) // P
                out = []
                for kt in range(nk):
                    rows = slice(kt * P, min((kt + 1) * P, p.shape[0]))
                    t = constp.tile(
                        [rows.stop - rows.start, p.shape[1]], p.dtype,
                        name=f"{p.name}_s{kt}",
                    )
                    nc.sync.dma_start(out=t[:], in_=p[rows, :])
                    out.append(t)
                return out

            xT_s = cload(xT)
            Wl1_s = cload_k(Wl1p)
            Wr1_s = cload_k(Wr1p)
            Wl2_s = cload_k(Wl2p)
            Wr2_s = cload_k(Wr2p)
            att1r_s = cload(att1r)
            att2r_s = cload(att2r)
            b1r_s = cload(b1r)
            b2r_s = cload(b2r)
            ident_s = cload(identp)
            crecip_s = cload(crecip)
            Wlin1_s = cload_k(Wlin1)
            blin1r_s = cload(blin1r)
            Wout_s = cload(Woutp)
            boutr_s = cload(boutr)
            ub_s = cload(ub)

            # ---- internal DRAM ----
            xl1_own = dram.tile([CAP, HC], bf)
            xr1_tab = dram.tile([CAP, HC], bf)
            xl1_ext = dram.tile([CAPEXT, HC], bf, addr_space="Shared")
            h1T = dram.tile([2, P, CAP], bf)
            xl2_own = dram.tile([CAP, HC], bf)
            xr2_tab = dram.tile([CAP, HC], bf)
            xl2_ext = dram.tile([CAPEXT, HC], bf, addr_space="Shared")
            gp_in = dram.tile([G, HC], f32)
            gp_out = dram.tile([G, HC], f32, addr_space="Shared")

            A_ = mybir.AluOpType
            AF = mybir.ActivationFunctionType

            # ================= node tables =================
            def node_tables(lhsT_tiles, W_s, tab):
                for b in range(NBLK):
                    ps = psA.tile([P, HC], f32, tag="a")
                    lts = lhsT_tiles(b)
                    assert len(lts) == len(W_s)
                    for i, lt in enumerate(lts):
                        nc.tensor.matmul(
                            ps[:], lhsT=lt, rhs=W_s[i][:],
                            start=(i == 0), stop=(i == len(lts) - 1),
                        )
                    ev = smallp.tile([P, HC], bf, tag="tabev")
                    nc.scalar.activation(out=ev[:], in_=ps[:], func=AF.Copy)
                    nc.sync.dma_start(out=tab[b * P : (b + 1) * P, :], in_=ev[:])

            def x_lhsT(b):
                return [xT_s[:, b * P : (b + 1) * P]]

            def h1_lhsT(b):
                outs = []
                for kt in range(2):
                    t = smallp.tile([P, P], bf, tag="h1l", name=f"h1l{b}_{kt}")
                    nc.sync.dma_start(out=t[:], in_=h1T[kt, :, b * P : (b + 1) * P])
                    outs.append(t[:])
                return outs

            # ================= edge pipeline =================
            qctr = [0]

            def edge_layer(xl_ext, xr_tab, attr_s, br_s, layer):
                gpool_ps = None
                if layer == 2:
                    gpool_ps = psG.tile([G, HC], f32, name=f"gpool_ps{layer}")
                for b in range(NBLK):
                    cols = slice(b * tb * P, (b + 1) * tb * P)
                    Ot_blk = metap.tile([P, tb * P], bf, tag="Ot")
                    nc.sync.dma_start(out=Ot_blk[:], in_=Otp[:, cols])
                    OtT_blk = metap.tile([P, tb * P], bf, tag="OtT")
                    nc.sync.dma_start(out=OtT_blk[:], in_=OtTp[:, cols])
                    ixa_t = metap.tile([P, tba * 8], dt.int16, tag="ixa")
                    nc.sync.dma_start(
                        out=ixa_t[:], in_=ixa[:, b * tba * 8 : (b + 1) * tba * 8]
                    )
                    ixb_t = metap.tile([P, tbb * 8], dt.int16, tag="ixb")
                    nc.sync.dma_start(
                        out=ixb_t[:], in_=ixb[:, b * tbb * 8 : (b + 1) * tbb * 8]
                    )
                    xr_blk = metap.tile([P, HC], bf, tag="xr")
                    nc.sync.dma_start(out=xr_blk[:], in_=xr_tab[b * P : (b + 1) * P, :])
                    xrb_blk = smallp.tile([P, HC], bf, tag="xrb")
                    nc.vector.tensor_tensor(
                        out=xrb_blk[:], in0=br_s[:], in1=xr_blk[:], op=A_.subtract
                    )

                    # ---- xl gathers (piece A tiles [0,tba), piece B [tba,tb)) ----
                    gxl = gbufp.tile([P, tb, HC], bf, tag="gxl")

                    def chunked_gather(dst_t0, n_tiles, table, idxt):
                        for q0 in range(0, n_tiles, CH):
                            q1 = min(q0 + CH, n_tiles)
                            nc.gpsimd.dma_gather(
                                out_ap=gxl[:, dst_t0 + q0 : dst_t0 + q1, :],
                                in_ap=table,
                                idxs_ap=idxt[:, q0 * 8 : q1 * 8],
                                num_idxs=(q1 - q0) * P, num_idxs_reg=(q1 - q0) * P,
                                elem_size=HC,
                                queue_num=qctr[0] % NQ,
                            )
                            qctr[0] += 1

                    chunked_gather(0, tba, xl_ext[0:MAXI16, :], ixa_t)
                    chunked_gather(tba, tbb, xl_ext[MAXI16:CAPEXT, :], ixb_t)

                    # ---- xr[dst] per edge via one-hot matmul ----
                    xre = gbufp.tile([P, tb, HC], bf, tag="xre")
                    for t0 in range(0, tb, 2):
                        k = min(2, tb - t0)
                        ps = psU.tile([P, 2, HC], f32, tag="u")
                        for u_ in range(k):
                            t_ = t0 + u_
                            nc.tensor.matmul(
                                ps[:, u_, :],
                                lhsT=OtT_blk[:, t_ * P : (t_ + 1) * P],
                                rhs=xr_blk[:], start=True, stop=True,
                            )
                        nc.scalar.activation(
                            out=xre[:, t0 : t0 + k, :], in_=ps[:, 0:k, :], func=AF.Copy
                        )

                    ut = workp.tile([P, tb, HC], bf, tag="ut")
                    nc.vector.tensor_tensor(out=ut[:], in0=gxl[:], in1=xre[:], op=A_.add)
                    ft = workp.tile([P, tb, HC], bf, tag="ft")
                    nc.scalar.activation(out=ft[:], in_=ut[:], func=AF.Prelu, alpha=0.2)
                    Pt = workp.tile([P, tb, HC], bf, tag="Pt")
                    nc.vector.tensor_tensor(
                        out=Pt[:], in0=ft[:], in1=_bcast_mid(attr_s[:], tb), op=A_.mult
                    )
                    v = Pt[:].rearrange("p t (h c) -> p (t h) c", h=H)
                    t1 = workp.tile([P, tb * H, 16], bf, tag="t1")
                    nc.vector.tensor_tensor(out=t1[:], in0=v[:, :, 0:16], in1=v[:, :, 16:32], op=A_.add)
                    t2 = workp.tile([P, tb * H, 8], bf, tag="t2")
                    nc.vector.tensor_tensor(out=t2[:], in0=t1[:, :, 0:8], in1=t1[:, :, 8:16], op=A_.add)
                    t3 = workp.tile([P, tb * H, 4], bf, tag="t3")
                    nc.vector.tensor_tensor(out=t3[:], in0=t2[:, :, 0:4], in1=t2[:, :, 4:8], op=A_.add)
                    t4 = workp.tile([P, tb * H, 2], bf, tag="t4")
                    nc.vector.tensor_tensor(out=t4[:], in0=t3[:, :, 0:2], in1=t3[:, :, 2:4], op=A_.add)
                    lg2 = workp.tile([P, tb * H, 2], bf, tag="lg2")
                    nc.vector.tensor_tensor(
                        out=lg2[:],
                        in0=t4[:, :, 0:1].broadcast_to([P, tb * H, 2]),
                        in1=t4[:, :, 1:2].broadcast_to([P, tb * H, 2]),
                        op=A_.add,
                    )
                    ex2 = workp.tile([P, tb * H, 2], bf, tag="ex2")
                    nc.scalar.activation(out=ex2[:], in_=lg2[:], func=AF.Exp)
                    msg = workp.tile([P, tb, HC], bf, tag="msg")
                    nc.vector.tensor_tensor(
                        out=msg[:].rearrange("p t (h k j) -> p (t h) k j", h=H, j=2),
                        in0=ut[:].rearrange("p t (h k j) -> p (t h) k j", h=H, j=2),
                        in1=ex2[:].unsqueeze(2).broadcast_to([P, tb * H, 16, 2]),
                        op=A_.mult,
                    )

                    psacc = psB.tile([P, HC + H], f32, tag="b")
                    acc = psacc[:, 0:HC]
                    accd = psacc[:, HC : HC + H]
                    exv = ex2[:].rearrange("p (t h) j -> p t h j", t=tb)
                    for t in range(tb):
                        Ot_t = Ot_blk[:, t * P : (t + 1) * P]
                        nc.tensor.matmul(
                            acc, lhsT=Ot_t, rhs=msg[:, t, :],
                            start=(t == 0), stop=(t == tb - 1),
                        )
                        # start=False even at t==0: acc's start=True already
                        # cleared the whole bank's has_written bits, so the
                        # first accd matmul overwrites (bit unset) rather than
                        # accumulating onto garbage; a second start=True here
                        # would re-clear the bank and drop acc's tile-0 sums.
                        nc.tensor.matmul(
                            accd, lhsT=Ot_t, rhs=exv[:, t, :, 0],
                            start=False, stop=(t == tb - 1),
                        )

                    denom = smallp.tile([P, H], f32, tag="denom")
                    nc.vector.tensor_scalar(
                        out=denom[:], in0=accd, scalar1=1e-20, scalar2=None,
                        op0=A_.max,
                    )
                    rec = smallp.tile([P, H], f32, tag="rec")
                    nc.vector.reciprocal(out=rec[:], in_=denom[:])
                    hsc = smallp.tile([P, HC], bf, tag="hsc")
                    nc.vector.tensor_tensor(
                        out=hsc[:].rearrange("p (h c) -> p h c", h=H),
                        in0=acc.rearrange("p (h c) -> p h c", h=H),
                        in1=rec[:].to_broadcast([P, H, C]),
                        op=A_.mult,
                    )
                    hfin = smallp.tile([P, HC], bf, tag="hfin")
                    nc.vector.tensor_tensor(out=hfin[:], in0=hsc[:], in1=xrb_blk[:], op=A_.add)
                    hout = smallp.tile([P, HC], bf, tag="hout")
                    nc.scalar.activation(out=hout[:], in_=hfin[:], func=AF.Relu)

                    if layer == 1:
                        for kt in range(2):
                            tp = psA.tile([P, P], bf, tag="a")
                            nc.tensor.transpose(
                                out=tp[:], in_=hout[:, kt * P : (kt + 1) * P],
                                identity=ident_s[:],
                            )
                            tps = smallp.tile([P, P], bf, tag="htps")
                            nc.scalar.activation(out=tps[:], in_=tp[:], func=AF.Copy)
                            nc.sync.dma_start(
                                out=h1T[kt, :, b * P : (b + 1) * P], in_=tps[:]
                            )
                    else:
                        gsel_blk = metap.tile([P, G], bf, tag="gselb")
                        nc.sync.dma_start(
                            out=gsel_blk[:], in_=gselp[b * P : (b + 1) * P, :]
                        )
                        nc.tensor.matmul(
                            gpool_ps[:], lhsT=gsel_blk[:], rhs=hout[:],
                            start=(b == 0), stop=(b == NBLK - 1),
                        )
                return gpool_ps

            # ================= layer 1 =================
            node_tables(x_lhsT, Wl1_s, xl1_own)
            nc.gpsimd.collective_compute(
                "AllGather", A_.bypass, replica_groups=groups,
                ins=[xl1_own.opt()], outs=[xl1_ext.opt()],
            )
            node_tables(x_lhsT, Wr1_s, xr1_tab)
            edge_layer(xl1_ext, xr1_tab, att1r_s, b1r_s, layer=1)

            # ================= layer 2 =================
            node_tables(h1_lhsT, Wl2_s, xl2_own)
            nc.gpsimd.collective_compute(
                "AllGather", A_.bypass, replica_groups=groups,
                ins=[xl2_own.opt()], outs=[xl2_ext.opt()],
            )
            node_tables(h1_lhsT, Wr2_s, xr2_tab)
            gpool_ps = edge_layer(xl2_ext, xr2_tab, att2r_s, b2r_s, layer=2)

            # ================= pool + MLP =================
            gsum = smallp.tile([G, HC], f32, tag="gsum")
            nc.scalar.activation(out=gsum[:], in_=gpool_ps[:], func=AF.Copy)
            nc.sync.dma_start(out=gp_in[:], in_=gsum[:])
            nc.gpsimd.collective_compute(
                "AllReduce", A_.add, replica_groups=groups,
                ins=[gp_in.opt()], outs=[gp_out.opt()],
            )
            gsum2 = smallp.tile([G, HC], f32, tag="gsum2")
            nc.sync.dma_start(out=gsum2[:], in_=gp_out[:])
            gmean = smallp.tile([G, HC], bf, tag="gmean")
            nc.vector.tensor_scalar(
                out=gmean[:], in0=gsum2[:], scalar1=crecip_s[:, 0:1], scalar2=None,
                op0=A_.mult,
            )
            gT = []
            for kt in range(2):
                tp = psA.tile([P, G], bf, tag="a")
                nc.tensor.transpose(
                    out=tp[:], in_=gmean[:, kt * P : (kt + 1) * P], identity=ident_s[:]
                )
                gkt = smallp.tile([P, G], bf, tag="gT", name=f"gT{kt}")
                nc.scalar.activation(out=gkt[:], in_=tp[:], func=AF.Copy)
                gT.append(gkt)
            lin_ps = psB.tile([G, 64], f32, tag="b")
            for kt in range(2):
                nc.tensor.matmul(
                    lin_ps[:], lhsT=gT[kt][:], rhs=Wlin1_s[kt][:],
                    start=(kt == 0), stop=(kt == 1),
                )
            lin = smallp.tile([G, 64], f32, tag="lin")
            nc.vector.tensor_tensor(out=lin[:], in0=lin_ps[:], in1=blin1r_s[:], op=A_.add)
            glu = smallp.tile([G, P], bf, tag="glu")
            nc.scalar.activation(out=glu[:, 0:64], in_=lin[:], func=AF.Relu)
            nc.vector.tensor_copy(out=glu[:, 64:67], in_=ub_s[:])
            nc.gpsimd.memset(glu[:, 67:P], 0.0)
            tp = psA.tile([P, G], bf, tag="a")
            nc.tensor.transpose(out=tp[:], in_=glu[:], identity=ident_s[:])
            gluT = smallp.tile([P, G], bf, tag="gluT")
            nc.scalar.activation(out=gluT[:], in_=tp[:], func=AF.Copy)
            out_ps = psB.tile([G, 1], f32, tag="b")
            nc.tensor.matmul(
                out_ps[:], lhsT=gluT[0:67, :], rhs=Wout_s[:], start=True, stop=True
            )
            outs = smallp.tile([G, 1], f32, tag="outs")
            nc.vector.tensor_tensor(out=outs[:], in0=out_ps[:], in1=boutr_s[:], op=A_.add)
            nc.sync.dma_start(out=out_g[:], in_=outs[:])

    nc.compile()
    _split_waits(nc)
    return nc


# ---------------------------------------------------------------------------
# Entry point
# ---------------------------------------------------------------------------


def kernel(**inputs):
    import os

    from concourse.bass_utils import run_bass_kernel_spmd

    x = np.asarray(inputs["x"], np.float32)
    edge_index = np.asarray(inputs["edge_index"], np.int64)
    batch = np.asarray(inputs["batch"], np.int64)
    u = np.asarray(inputs["u"], np.float32)
    weights = {
        k: np.asarray(inputs[k], np.float32)
        for k in ("Wl1", "Wr1", "att1", "b1", "Wl2", "Wr2", "att2", "b2",
                  "W_lin1", "b_lin1", "W_out", "b_out")
    }
    percore, row_of, cfg = _plan_blocks(edge_index)
    maps = _prep(x, batch, u, weights, cfg, percore, row_of)
    nc = _build(cfg, in_dim=x.shape[1])
    trace = bool(os.environ.get("KERNEL_TRACE"))
    try:
        res = run_bass_kernel_spmd(nc, maps, list(range(NCORES)), trace=trace)
    except ModuleNotFoundError:
        res = run_bass_kernel_spmd(nc, maps, list(range(NCORES)))
    if trace and getattr(res, "exec_time_ns", None) is not None:
        print(f"HW exec time: {res.exec_time_ns} ns")
    return res.results[0]["out_g"].reshape(G).astype(np.float32)


# revision 10
# speedup vs baseline: 2.7749x; 1.0819x over previous
"""BrainAgeGAT Trainium2 kernel: 2-layer GATv2 + mean-pool + MLP on 8 NeuronCores.

Strategy (v2):
  - Edges (incl. self loops) sharded by destination across the 8 cores; within
    a core, destination nodes are LPT-packed into 50 blocks of <=127 slots
    (slot 127 = garbage) so per-block edge counts are balanced and every block
    uses the same uniform tile counts (tba piece-A tiles + tbb piece-B tiles).
  - xl = x@Wl is AllGather'd; per edge a 512-byte bf16 row is fetched with
    dma_gather (SWDGE, int16 indices; the 51200-row table is split in two
    25600-row halves to stay within int16).
  - xr[dst] is NOT gathered: per block the 128-row xr slice is SBUF-resident
    and expanded per edge with a one-hot matmul (lhsT = OtT streamed from
    host) into PSUM, then copied to SBUF by ACT. The same one-hot (untransposed
    Ot, also host-streamed) drives the scatter-sum matmuls.
  - u = xl[src]+xr[dst] on DVE; logits = per-head tree-reduction of
    att * leaky_relu(u) (ACT Prelu + DVE); softmax needs no max subtraction at
    these magnitudes. Since softmax weights sum to 1, we scatter exp(logit)*u
    and subtract xr once per destination at the end. exp runs on the tiny
    [P, tb*H, 2] logit pair (not the 32x broadcast).
  - Mean-pool via per-block one-hot graph-selector matmuls into a persistent
    PSUM accumulator, an 8-core AllReduce, and a tiny MLP.
"""

import math
import sys

sys.path.insert(0, "/opt/trn_rl_repo")

import ml_dtypes
import numpy as np

import concourse.bacc as bacc
import concourse.bass as bass
import concourse.mybir as mybir
import concourse.tile as tile
from concourse import library_config
from concourse.vector_clock import ScopedClock

BF16 = ml_dtypes.bfloat16

# ---------------------------------------------------------------------------
# Patches for walrus' one-sync-wait-per-instruction limit.
# ---------------------------------------------------------------------------


def _drain_and_barrier(self, tick_clock, wait_clock):
    nc = self.nc
    probe = nc.sync.nop(nofuse=True, hint="drain_wait_split")
    wait_clock.add_sem_waits(probe.ins, ScopedClock({None: tick_clock.global_clock}))
    si = probe.ins.sync_info
    waits = list(si.on_wait) if si and si.on_wait else []
    if len(waits) > 1:
        si.on_wait = waits[:1]
        for w in waits[1:]:
            extra = nc.sync.nop(nofuse=True, hint="drain_wait_split")
            extra.ins.sync_info = type(si)(on_wait=[w], on_update=[])
    nc.sync.drain()
    nc.all_engine_barrier()
    assert self.sems is not None
    popped = nc._tile_sem_poison_stack.pop()
    assert popped is self._sem_poison
    nc.clear_and_free_semaphores(list(self.sems.allocated().values()))
    nc.all_engine_barrier()


tile.TileContext._drain_and_barrier = _drain_and_barrier


def _split_waits(nc):
    """walrus codegen accepts one sync-wait command per instruction; Tile can
    emit several. Hoist extras onto preceding same-engine NoOps."""
    for bb in nc.main_func.blocks:
        out = []
        for ins in bb.instructions:
            si = ins.sync_info
            waits = list(si.on_wait) if si and si.on_wait else []
            if len(waits) > 1:
                for w in waits[:-1]:
                    nop = mybir.InstNoOp(
                        name=nc.get_next_instruction_name(), ins=[], outs=[]
                    )
                    nop.engine = ins.engine
                    nop.sync_info = mybir.SyncInfo(on_wait=[w], on_update=[])
                    nc.register_instruction(nop)
                    out.append(nop)
                si.on_wait = [waits[-1]]
            out.append(ins)
        bb.instructions = out


# ---------------------------------------------------------------------------
# Model dimensions (hardcoded per problem spec)
# ---------------------------------------------------------------------------
N = 50000
E = 800000
G = 128
H = 8
C = 32
HC = H * C  # 256
P = 128
NCORES = 8
SLOTS = 127  # real slots per block (slot 127 = garbage)
MAXI16 = 25600  # table-piece size for int16 gather indices
NPC = N // NCORES  # 6250
NBLK = (NPC + SLOTS - 1) // SLOTS  # 50
CAP = NBLK * P  # 6400
CAPEXT = NCORES * CAP  # 51200
CH = 5  # gather tiles per dma_gather call
NQ = 4  # SWDGE queues to rotate gathers over


class Cfg:
    def __init__(self, tba, tbb):
        self.tba = tba
        self.tbb = tbb
        self.tb = tba + tbb
        self.ttot = NBLK * self.tb


# ---------------------------------------------------------------------------
# Host-side preprocessing
# ---------------------------------------------------------------------------


def _f32(a):
    return np.ascontiguousarray(a, dtype=np.float32)


def _bf(a):
    return np.ascontiguousarray(np.asarray(a, dtype=np.float32).astype(BF16))


def _wrap_idx(ids):
    """Gather-index list -> [128, len/16] int16 in the SWDGE wrap layout
    (idx j read from [j % 16, j // 16], replicated over the 8 Q7 cores)."""
    ids = np.asarray(ids, np.int16)
    assert len(ids) % 16 == 0
    w = ids.reshape(-1, 16).T  # [16, s]
    return np.tile(w, (8, 1))  # [128, s]


def _plan_blocks(edge_index):
    """LPT-pack dst nodes into blocks; return assignment + per-core edge
    structures + uniform tile counts."""
    src = np.concatenate([edge_index[0], np.arange(N)]).astype(np.int64)
    dst = np.concatenate([edge_index[1], np.arange(N)]).astype(np.int64)
    pieceB = (src // NPC) >= (NCORES // 2)
    dega = np.bincount(dst[~pieceB], minlength=N)
    degb = np.bincount(dst[pieceB], minlength=N)

    blk_of = np.empty(N, np.int64)
    slot_of = np.empty(N, np.int64)
    for c in range(NCORES):
        lo = c * NPC
        da = dega[lo : lo + NPC].astype(np.float64)
        db = degb[lo : lo + NPC].astype(np.float64)
        order = np.argsort(-(da + db), kind="stable")
        blk_a = np.zeros(NBLK)
        blk_b = np.zeros(NBLK)
        blk_n = np.zeros(NBLK, np.int64)
        for i in order:
            cost = np.maximum(blk_a + da[i], blk_b + db[i])
            cost[blk_n >= SLOTS] = np.inf
            j = int(np.argmin(cost))
            blk_of[lo + i] = j
            slot_of[lo + i] = blk_n[j]
            blk_a[j] += da[i]
            blk_b[j] += db[i]
            blk_n[j] += 1
    row_of = blk_of * P + slot_of  # within-core table row
    ext_row = (np.arange(N) // NPC) * CAP + row_of  # global table row

    # per-(core, block, piece) edge lists
    esrow = ext_row[src]
    eslot = slot_of[dst]
    eblk = blk_of[dst]
    ecore = dst // NPC
    percore = []
    na = np.zeros((NCORES, NBLK), int)
    nb_ = np.zeros((NCORES, NBLK), int)
    for c in range(NCORES):
        blocks = []
        selc = ecore == c
        for b in range(NBLK):
            sel = selc & (eblk == b)
            sa = sel & ~pieceB
            sb = sel & pieceB
            ra, la = esrow[sa], eslot[sa]
            rb, lb = esrow[sb] - MAXI16, eslot[sb]
            blocks.append((ra, la, rb, lb))
            na[c, b] = len(ra)
            nb_[c, b] = len(rb)
        percore.append(blocks)
    tba = int(math.ceil(na.max() / P))
    tbb = int(math.ceil(nb_.max() / P))
    return percore, row_of, Cfg(tba, tbb)


def _prep(x, batch, u, weights, cfg: Cfg, percore, row_of):
    att1 = weights["att1"]
    att2 = weights["att2"]

    def att_rep(att):
        return _bf(np.broadcast_to(att.reshape(-1), (P, HC)))

    tba, tbb, tb = cfg.tba, cfg.tbb, cfg.tb
    iota = np.arange(P)

    maps = []
    for c in range(NCORES):
        m = {}
        lo = c * NPC
        ixa = np.zeros((P, NBLK * tba * 8), np.int16)
        ixb = np.zeros((P, NBLK * tbb * 8), np.int16)
        Ot = np.zeros((P, NBLK * tb * P), BF16)
        OtT = np.zeros((P, NBLK * tb * P), BF16)
        for b in range(NBLK):
            ra, la, rb, lb = percore[c][b]
            ia = np.zeros(tba * P, np.int64)
            ia[: len(ra)] = ra
            ib = np.zeros(tbb * P, np.int64)
            ib[: len(rb)] = rb
            ixa[:, b * tba * 8 : (b + 1) * tba * 8] = _wrap_idx(ia)
            ixb[:, b * tbb * 8 : (b + 1) * tbb * 8] = _wrap_idx(ib)
            slots = np.full(tb * P, 127, np.int64)
            slots[: len(la)] = la
            slots[tba * P : tba * P + len(lb)] = lb
            oh = (slots[:, None] == iota[None, :]).astype(BF16)  # [tb*P, P]
            oh = oh.reshape(tb, P, P)
            cols = slice(b * tb * P, (b + 1) * tb * P)
            Ot[:, cols] = oh.transpose(1, 0, 2).reshape(P, tb * P)
            OtT[:, cols] = oh.transpose(2, 0, 1).reshape(P, tb * P)
        m["ixa"] = ixa
        m["ixb"] = ixb
        m["Ot"] = np.ascontiguousarray(Ot)
        m["OtT"] = np.ascontiguousarray(OtT)

        rows = row_of[lo : lo + NPC]
        xs = np.zeros((CAP, x.shape[1]), np.float32)
        xs[rows] = x[lo : lo + NPC]
        m["xT"] = _bf(xs.T)

        gsel = np.zeros((CAP, G), np.float32)
        gsel[rows, np.asarray(batch[lo : lo + NPC])] = 1.0
        m["gsel"] = _bf(gsel)
        maps.append(m)

    counts = np.bincount(np.asarray(batch), minlength=G).astype(np.float32)
    shared = {
        "Wl1": _bf(weights["Wl1"]),
        "Wr1": _bf(weights["Wr1"]),
        "Wl2": _bf(weights["Wl2"]),
        "Wr2": _bf(weights["Wr2"]),
        "att1r": att_rep(att1),
        "att2r": att_rep(att2),
        "b1r": _bf(np.broadcast_to(weights["b1"], (P, HC))),
        "b2r": _bf(np.broadcast_to(weights["b2"], (P, HC))),
        "ident": _bf(np.eye(P, dtype=np.float32)),
        "crecip": _f32((1.0 / np.maximum(counts, 1.0)).reshape(G, 1)),
        "Wlin1": _bf(weights["W_lin1"]),
        "blin1r": _f32(np.broadcast_to(weights["b_lin1"], (G, 64))),
        "Wout": _bf(weights["W_out"]),
        "boutr": _f32(np.full((G, 1), float(weights["b_out"][0]), np.float32)),
        "ub": _bf(u),
    }
    for m in maps:
        m.update(shared)
    return maps


# ---------------------------------------------------------------------------
# Device program
# ---------------------------------------------------------------------------


def _bcast_mid(ap, reps):
    return ap.unsqueeze(1).broadcast_to([ap.shape[0], reps, ap.shape[1]])


def _build(cfg: Cfg, in_dim=3):
    dt = mybir.dt
    bf = dt.bfloat16
    f32 = dt.float32
    nc = bacc.Bacc(None, num_swdge_queues=NQ) if NQ > 1 else bacc.Bacc(None)
    groups = [list(range(NCORES))]
    tba, tbb, tb = cfg.tba, cfg.tbb, cfg.tb

    def prm(name, shape, dtype):
        return nc.declare_dram_parameter(name, list(shape), dtype, isOutput=False)

    xT = prm("xT", [in_dim, CAP], bf)
    ixa = prm("ixa", [P, NBLK * tba * 8], dt.int16)
    ixb = prm("ixb", [P, NBLK * tbb * 8], dt.int16)
    Otp = prm("Ot", [P, NBLK * tb * P], bf)
    OtTp = prm("OtT", [P, NBLK * tb * P], bf)
    Wl1p = prm("Wl1", [in_dim, HC], bf)
    Wr1p = prm("Wr1", [in_dim, HC], bf)
    Wl2p = prm("Wl2", [HC, HC], bf)
    Wr2p = prm("Wr2", [HC, HC], bf)
    att1r = prm("att1r", [P, HC], bf)
    att2r = prm("att2r", [P, HC], bf)
    b1r = prm("b1r", [P, HC], bf)
    b2r = prm("b2r", [P, HC], bf)
    identp = prm("ident", [P, P], bf)
    gselp = prm("gsel", [CAP, G], bf)
    crecip = prm("crecip", [G, 1], f32)
    Wlin1 = prm("Wlin1", [HC, 64], bf)
    blin1r = prm("blin1r", [G, 64], f32)
    Woutp = prm("Wout", [64 + 3, 1], bf)
    boutr = prm("boutr", [G, 1], f32)
    ub = prm("ub", [G, 3], bf)
    out_g = nc.declare_dram_parameter("out_g", [G, 1], f32, isOutput=True)

    with tile.TileContext(nc) as tc:
        with (
            tc.tile_pool(name="const", bufs=1) as constp,
            tc.tile_pool(name="meta", bufs=2) as metap,
            tc.tile_pool(name="gbuf", bufs=2) as gbufp,
            tc.tile_pool(name="work", bufs=2) as workp,
            tc.tile_pool(name="small", bufs=3) as smallp,
            tc.tile_pool(name="psU", bufs=2, space="PSUM") as psU,
            tc.tile_pool(name="psA", bufs=2, space="PSUM") as psA,
            tc.tile_pool(name="psB", bufs=2, space="PSUM") as psB,
            tc.tile_pool(name="psG", bufs=1, space="PSUM") as psG,
            tc.tile_pool(name="dram", bufs=1, space="DRAM") as dram,
        ):
            # ---- constants to SBUF ----
            def cload(p):
                t = constp.tile([p.shape[0], p.shape[1]], p.dtype, name=p.name + "_s")
                nc.sync.dma_start(out=t[:], in_=p[:])
                return t

            def cload_k(p):
                nk = (p.shape[0] + P - 1) // P
                out = []
                for kt in range(nk):
                    rows = slice(kt * P, min((kt + 1) * P, p.shape[0]))
                    t = constp.tile(
                        [rows.stop - rows.start, p.shape[1]], p.dtype,
                        name=f"{p.name}_s{kt}",
                    )
                    nc.sync.dma_start(out=t[:], in_=p[rows, :])
                    out.append(t)
                return out

            xT_s = cload(xT)
            Wl1_s = cload_k(Wl1p)
            Wr1_s = cload_k(Wr1p)
            Wl2_s = cload_k(Wl2p)
            Wr2_s = cload_k(Wr2p)
            att1r_s = cload(att1r)
            att2r_s = cload(att2r)
            b1r_s = cload(b1r)
            b2r_s = cload(b2r)
            ident_s = cload(identp)
            crecip_s = cload(crecip)
            Wlin1_s = cload_k(Wlin1)
            blin1r_s = cload(blin1r)
            Wout_s = cload(Woutp)
            boutr_s = cload(boutr)
            ub_s = cload(ub)

            # ---- internal DRAM ----
            xl1_own = dram.tile([CAP, HC], bf)
            xr1_tab = dram.tile([CAP, HC], bf)
            xl1_ext = dram.tile([CAPEXT, HC], bf, addr_space="Shared")
            xl2_own = dram.tile([CAP, HC], bf)
            xr2_tab = dram.tile([CAP, HC], bf)
            xl2_ext = dram.tile([CAPEXT, HC], bf, addr_space="Shared")
            gp_in = dram.tile([G, HC], f32)
            gp_out = dram.tile([G, HC], f32, addr_space="Shared")

            A_ = mybir.AluOpType
            AF = mybir.ActivationFunctionType

            # ================= node tables =================
            def node_tables(lhsT_tiles, W_s, tab):
                for b in range(NBLK):
                    ps = psA.tile([P, HC], f32, tag="a")
                    lts = lhsT_tiles(b)
                    assert len(lts) == len(W_s)
                    for i, lt in enumerate(lts):
                        nc.tensor.matmul(
                            ps[:], lhsT=lt, rhs=W_s[i][:],
                            start=(i == 0), stop=(i == len(lts) - 1),
                        )
                    ev = smallp.tile([P, HC], bf, tag="tabev")
                    nc.scalar.activation(out=ev[:], in_=ps[:], func=AF.Copy)
                    nc.sync.dma_start(out=tab[b * P : (b + 1) * P, :], in_=ev[:])

            def x_lhsT(b):
                return [xT_s[:, b * P : (b + 1) * P]]

            # ================= edge pipeline =================
            qctr = [0]

            def edge_layer(xl_ext, xr_tab, attr_s, br_s, layer):
                gpool_ps = None
                if layer == 2:
                    gpool_ps = psG.tile([G, HC], f32, name=f"gpool_ps{layer}")

                def front(b):
                    """Block prologue: meta loads, gathers, one-hot xr expand.
                    Emitted one block ahead so these fill other blocks' stalls."""
                    st = {}
                    cols = slice(b * tb * P, (b + 1) * tb * P)
                    Ot_blk = metap.tile([P, tb * P], bf, tag="Ot")
                    nc.sync.dma_start(out=Ot_blk[:], in_=Otp[:, cols])
                    OtT_blk = metap.tile([P, tb * P], bf, tag="OtT")
                    nc.sync.dma_start(out=OtT_blk[:], in_=OtTp[:, cols])
                    ixa_t = metap.tile([P, tba * 8], dt.int16, tag="ixa")
                    nc.sync.dma_start(
                        out=ixa_t[:], in_=ixa[:, b * tba * 8 : (b + 1) * tba * 8]
                    )
                    ixb_t = metap.tile([P, tbb * 8], dt.int16, tag="ixb")
                    nc.sync.dma_start(
                        out=ixb_t[:], in_=ixb[:, b * tbb * 8 : (b + 1) * tbb * 8]
                    )
                    xr_blk = metap.tile([P, HC], bf, tag="xr")
                    nc.sync.dma_start(out=xr_blk[:], in_=xr_tab[b * P : (b + 1) * P, :])
                    if layer == 2:
                        gsel_blk = metap.tile([P, G], bf, tag="gselb")
                        nc.sync.dma_start(
                            out=gsel_blk[:], in_=gselp[b * P : (b + 1) * P, :]
                        )
                        st["gsel"] = gsel_blk
                    xrb_blk = smallp.tile([P, HC], bf, tag="xrb")
                    nc.vector.tensor_tensor(
                        out=xrb_blk[:], in0=br_s[:], in1=xr_blk[:], op=A_.subtract
                    )

                    gxl = gbufp.tile([P, tb, HC], bf, tag="gxl")

                    def chunked_gather(dst_t0, n_tiles, table, idxt):
                        for q0 in range(0, n_tiles, CH):
                            q1 = min(q0 + CH, n_tiles)
                            nc.gpsimd.dma_gather(
                                out_ap=gxl[:, dst_t0 + q0 : dst_t0 + q1, :],
                                in_ap=table,
                                idxs_ap=idxt[:, q0 * 8 : q1 * 8],
                                num_idxs=(q1 - q0) * P, num_idxs_reg=(q1 - q0) * P,
                                elem_size=HC,
                                queue_num=qctr[0] % NQ,
                            )
                            qctr[0] += 1

                    chunked_gather(0, tba, xl_ext[0:MAXI16, :], ixa_t)
                    chunked_gather(tba, tbb, xl_ext[MAXI16:CAPEXT, :], ixb_t)

                    xre = gbufp.tile([P, tb, HC], bf, tag="xre")
                    for t0 in range(0, tb, 2):
                        k = min(2, tb - t0)
                        ps = psU.tile([P, 2, HC], f32, tag="u")
                        for u_ in range(k):
                            t_ = t0 + u_
                            nc.tensor.matmul(
                                ps[:, u_, :],
                                lhsT=OtT_blk[:, t_ * P : (t_ + 1) * P],
                                rhs=xr_blk[:], start=True, stop=True,
                            )
                        if t0 == 0:
                            nc.vector.tensor_copy(
                                out=xre[:, t0 : t0 + k, :], in_=ps[:, 0:k, :]
                            )
                        else:
                            nc.scalar.activation(
                                out=xre[:, t0 : t0 + k, :], in_=ps[:, 0:k, :],
                                func=AF.Copy,
                            )
                    st.update(Ot=Ot_blk, gxl=gxl, xre=xre, xrb=xrb_blk)
                    return st

                def back(b, st):
                    Ot_blk, gxl, xre, xrb_blk = st["Ot"], st["gxl"], st["xre"], st["xrb"]
                    ut = workp.tile([P, tb, HC], bf, tag="ut")
                    nc.vector.tensor_tensor(out=ut[:], in0=gxl[:], in1=xre[:], op=A_.add)
                    ft = workp.tile([P, tb, HC], bf, tag="ft")
                    nc.scalar.activation(out=ft[:], in_=ut[:], func=AF.Prelu, alpha=0.2)
                    Pt = workp.tile([P, tb, HC], bf, tag="Pt")
                    nc.vector.tensor_tensor(
                        out=Pt[:], in0=ft[:], in1=_bcast_mid(attr_s[:], tb), op=A_.mult
                    )
                    v = Pt[:].rearrange("p t (h c) -> p (t h) c", h=H)
                    t1 = workp.tile([P, tb * H, 16], bf, tag="t1")
                    nc.vector.tensor_tensor(out=t1[:], in0=v[:, :, 0:16], in1=v[:, :, 16:32], op=A_.add)
                    t2 = workp.tile([P, tb * H, 8], bf, tag="t2")
                    nc.vector.tensor_tensor(out=t2[:], in0=t1[:, :, 0:8], in1=t1[:, :, 8:16], op=A_.add)
                    t3 = workp.tile([P, tb * H, 4], bf, tag="t3")
                    nc.vector.tensor_tensor(out=t3[:], in0=t2[:, :, 0:4], in1=t2[:, :, 4:8], op=A_.add)
                    t4 = workp.tile([P, tb * H, 2], bf, tag="t4")
                    nc.vector.tensor_tensor(out=t4[:], in0=t3[:, :, 0:2], in1=t3[:, :, 2:4], op=A_.add)
                    lg2 = workp.tile([P, tb * H, 2], bf, tag="lg2")
                    nc.vector.tensor_tensor(
                        out=lg2[:],
                        in0=t4[:, :, 0:1].broadcast_to([P, tb * H, 2]),
                        in1=t4[:, :, 1:2].broadcast_to([P, tb * H, 2]),
                        op=A_.add,
                    )
                    ex2 = workp.tile([P, tb * H, 2], bf, tag="ex2")
                    nc.scalar.activation(out=ex2[:], in_=lg2[:], func=AF.Exp)
                    exv = ex2[:].rearrange("p (t h) j -> p t h j", t=tb)
                    msg = workp.tile([P, tb, HC], bf, tag="msg")
                    nc.vector.tensor_tensor(
                        out=msg[:].rearrange("p t (h k j) -> p (t h) k j", h=H, j=2),
                        in0=ut[:].rearrange("p t (h k j) -> p (t h) k j", h=H, j=2),
                        in1=ex2[:].unsqueeze(2).broadcast_to([P, tb * H, 16, 2]),
                        op=A_.mult,
                    )

                    psacc = psB.tile([P, HC + H], f32, tag="b")
                    acc = psacc[:, 0:HC]
                    accd = psacc[:, HC : HC + H]
                    for t in range(tb):
                        Ot_t = Ot_blk[:, t * P : (t + 1) * P]
                        nc.tensor.matmul(
                            acc, lhsT=Ot_t, rhs=msg[:, t, :],
                            start=(t == 0), stop=(t == tb - 1),
                        )
                        # start=False even at t==0: acc's start=True already
                        # cleared the whole bank's has_written bits, so the
                        # first accd matmul overwrites (bit unset) rather than
                        # accumulating onto garbage; a second start=True here
                        # would re-clear the bank and drop acc's tile-0 sums.
                        nc.tensor.matmul(
                            accd, lhsT=Ot_t, rhs=exv[:, t, :, 0],
                            start=False, stop=(t == tb - 1),
                        )

                    denom = smallp.tile([P, H], f32, tag="denom")
                    nc.vector.tensor_scalar(
                        out=denom[:], in0=accd, scalar1=1e-20, scalar2=None,
                        op0=A_.max,
                    )
                    rec = smallp.tile([P, H], f32, tag="rec")
                    nc.vector.reciprocal(out=rec[:], in_=denom[:])
                    hsc = smallp.tile([P, HC], bf, tag="hsc")
                    nc.vector.tensor_tensor(
                        out=hsc[:].rearrange("p (h c) -> p h c", h=H),
                        in0=acc.rearrange("p (h c) -> p h c", h=H),
                        in1=rec[:].to_broadcast([P, H, C]),
                        op=A_.mult,
                    )
                    hfin = smallp.tile([P, HC], bf, tag="hfin")
                    nc.vector.tensor_tensor(out=hfin[:], in0=hsc[:], in1=xrb_blk[:], op=A_.add)
                    hout = smallp.tile([P, HC], bf, tag="hout")
                    nc.scalar.activation(out=hout[:], in_=hfin[:], func=AF.Relu)

                    if layer == 1:
                        # transpose h1 block and compute layer-2 node tables
                        # inline (h1T never round-trips through DRAM).
                        tps = []
                        for kt in range(2):
                            tp = psA.tile([P, P], bf, tag="a")
                            nc.tensor.transpose(
                                out=tp[:], in_=hout[:, kt * P : (kt + 1) * P],
                                identity=ident_s[:],
                            )
                            t_ = smallp.tile([P, P], bf, tag="htps")
                            nc.scalar.activation(out=t_[:], in_=tp[:], func=AF.Copy)
                            tps.append(t_)
                        for W_s, tab in ((Wl2_s, xl2_own), (Wr2_s, xr2_tab)):
                            ps2 = psA.tile([P, HC], f32, tag="a")
                            for i in range(2):
                                nc.tensor.matmul(
                                    ps2[:], lhsT=tps[i][:], rhs=W_s[i][:],
                                    start=(i == 0), stop=(i == 1),
                                )
                            ev = smallp.tile([P, HC], bf, tag="tabev")
                            nc.scalar.activation(out=ev[:], in_=ps2[:], func=AF.Copy)
                            nc.sync.dma_start(
                                out=tab[b * P : (b + 1) * P, :], in_=ev[:]
                            )
                    else:
                        nc.tensor.matmul(
                            gpool_ps[:], lhsT=st["gsel"][:], rhs=hout[:],
                            start=(b == 0), stop=(b == NBLK - 1),
                        )

                st = front(0)
                for b in range(NBLK):
                    nxt = front(b + 1) if b + 1 < NBLK else None
                    back(b, st)
                    st = nxt
                return gpool_ps

            # ================= layer 1 =================
            node_tables(x_lhsT, Wl1_s, xl1_own)
            nc.gpsimd.collective_compute(
                "AllGather", A_.bypass, replica_groups=groups,
                ins=[xl1_own.opt()], outs=[xl1_ext.opt()],
            )
            node_tables(x_lhsT, Wr1_s, xr1_tab)
            edge_layer(xl1_ext, xr1_tab, att1r_s, b1r_s, layer=1)

            # ================= layer 2 =================
            nc.gpsimd.collective_compute(
                "AllGather", A_.bypass, replica_groups=groups,
                ins=[xl2_own.opt()], outs=[xl2_ext.opt()],
            )
            gpool_ps = edge_layer(xl2_ext, xr2_tab, att2r_s, b2r_s, layer=2)

            # ================= pool + MLP =================
            gsum = smallp.tile([G, HC], f32, tag="gsum")
            nc.scalar.activation(out=gsum[:], in_=gpool_ps[:], func=AF.Copy)
            nc.sync.dma_start(out=gp_in[:], in_=gsum[:])
            nc.gpsimd.collective_compute(
                "AllReduce", A_.add, replica_groups=groups,
                ins=[gp_in.opt()], outs=[gp_out.opt()],
            )
            gsum2 = smallp.tile([G, HC], f32, tag="gsum2")
            nc.sync.dma_start(out=gsum2[:], in_=gp_out[:])
            gmean = smallp.tile([G, HC], bf, tag="gmean")
            nc.vector.tensor_scalar(
                out=gmean[:], in0=gsum2[:], scalar1=crecip_s[:, 0:1], scalar2=None,
                op0=A_.mult,
            )
            gT = []
            for kt in range(2):
                tp = psA.tile([P, G], bf, tag="a")
                nc.tensor.transpose(
                    out=tp[:], in_=gmean[:, kt * P : (kt + 1) * P], identity=ident_s[:]
                )
                gkt = smallp.tile([P, G], bf, tag="gT", name=f"gT{kt}")
                nc.scalar.activation(out=gkt[:], in_=tp[:], func=AF.Copy)
                gT.append(gkt)
            lin_ps = psB.tile([G, 64], f32, tag="b")
            for kt in range(2):
                nc.tensor.matmul(
                    lin_ps[:], lhsT=gT[kt][:], rhs=Wlin1_s[kt][:],
                    start=(kt == 0), stop=(kt == 1),
                )
            lin = smallp.tile([G, 64], f32, tag="lin")
            nc.vector.tensor_tensor(out=lin[:], in0=lin_ps[:], in1=blin1r_s[:], op=A_.add)
            glu = smallp.tile([G, P], bf, tag="glu")
            nc.scalar.activation(out=glu[:, 0:64], in_=lin[:], func=AF.Relu)
            nc.vector.tensor_copy(out=glu[:, 64:67], in_=ub_s[:])
            nc.gpsimd.memset(glu[:, 67:P], 0.0)
            tp = psA.tile([P, G], bf, tag="a")
            nc.tensor.transpose(out=tp[:], in_=glu[:], identity=ident_s[:])
            gluT = smallp.tile([P, G], bf, tag="gluT")
            nc.scalar.activation(out=gluT[:], in_=tp[:], func=AF.Copy)
            out_ps = psB.tile([G, 1], f32, tag="b")
            nc.tensor.matmul(
                out_ps[:], lhsT=gluT[0:67, :], rhs=Wout_s[:], start=True, stop=True
            )
            outs = smallp.tile([G, 1], f32, tag="outs")
            nc.vector.tensor_tensor(out=outs[:], in0=out_ps[:], in1=boutr_s[:], op=A_.add)
            nc.sync.dma_start(out=out_g[:], in_=outs[:])

    nc.compile()
    _split_waits(nc)
    return nc


# ---------------------------------------------------------------------------
# Entry point
# ---------------------------------------------------------------------------


def kernel(**inputs):
    import os

    from concourse.bass_utils import run_bass_kernel_spmd

    x = np.asarray(inputs["x"], np.float32)
    edge_index = np.asarray(inputs["edge_index"], np.int64)
    batch = np.asarray(inputs["batch"], np.int64)
    u = np.asarray(inputs["u"], np.float32)
    weights = {
        k: np.asarray(inputs[k], np.float32)
        for k in ("Wl1", "Wr1", "att1", "b1", "Wl2", "Wr2", "att2", "b2",
                  "W_lin1", "b_lin1", "W_out", "b_out")
    }
    percore, row_of, cfg = _plan_blocks(edge_index)
    maps = _prep(x, batch, u, weights, cfg, percore, row_of)
    nc = _build(cfg, in_dim=x.shape[1])
    trace = bool(os.environ.get("KERNEL_TRACE"))
    try:
        res = run_bass_kernel_spmd(nc, maps, list(range(NCORES)), trace=trace)
    except ModuleNotFoundError:
        res = run_bass_kernel_spmd(nc, maps, list(range(NCORES)))
    if trace and getattr(res, "exec_time_ns", None) is not None:
        print(f"HW exec time: {res.exec_time_ns} ns")
    return res.results[0]["out_g"].reshape(G).astype(np.float32)


# revision 12
# speedup vs baseline: 2.8198x; 1.0162x over previous
"""BrainAgeGAT Trainium2 kernel: 2-layer GATv2 + mean-pool + MLP on 8 NeuronCores.

Strategy (v2):
  - Edges (incl. self loops) sharded by destination across the 8 cores; within
    a core, destination nodes are LPT-packed into 50 blocks of <=127 slots
    (slot 127 = garbage) so per-block edge counts are balanced and every block
    uses the same uniform tile counts (tba piece-A tiles + tbb piece-B tiles).
  - xl = x@Wl is AllGather'd; per edge a 512-byte bf16 row is fetched with
    dma_gather (SWDGE, int16 indices; the 51200-row table is split in two
    25600-row halves to stay within int16).
  - xr[dst] is NOT gathered: per block the 128-row xr slice is SBUF-resident
    and expanded per edge with a one-hot matmul (lhsT = OtT streamed from
    host) into PSUM, then copied to SBUF by ACT. The same one-hot (untransposed
    Ot, also host-streamed) drives the scatter-sum matmuls.
  - u = xl[src]+xr[dst] on DVE; logits = per-head tree-reduction of
    att * leaky_relu(u) (ACT Prelu + DVE); softmax needs no max subtraction at
    these magnitudes. Since softmax weights sum to 1, we scatter exp(logit)*u
    and subtract xr once per destination at the end. exp runs on the tiny
    [P, tb*H, 2] logit pair (not the 32x broadcast).
  - Mean-pool via per-block one-hot graph-selector matmuls into a persistent
    PSUM accumulator, an 8-core AllReduce, and a tiny MLP.
"""

import math
import sys

sys.path.insert(0, "/opt/trn_rl_repo")

import ml_dtypes
import numpy as np

import concourse.bacc as bacc
import concourse.bass as bass
import concourse.mybir as mybir
import concourse.tile as tile
from concourse import library_config
from concourse.vector_clock import ScopedClock

BF16 = ml_dtypes.bfloat16

# ---------------------------------------------------------------------------
# Patches for walrus' one-sync-wait-per-instruction limit.
# ---------------------------------------------------------------------------


def _drain_and_barrier(self, tick_clock, wait_clock):
    nc = self.nc
    probe = nc.sync.nop(nofuse=True, hint="drain_wait_split")
    wait_clock.add_sem_waits(probe.ins, ScopedClock({None: tick_clock.global_clock}))
    si = probe.ins.sync_info
    waits = list(si.on_wait) if si and si.on_wait else []
    if len(waits) > 1:
        si.on_wait = waits[:1]
        for w in waits[1:]:
            extra = nc.sync.nop(nofuse=True, hint="drain_wait_split")
            extra.ins.sync_info = type(si)(on_wait=[w], on_update=[])
    nc.sync.drain()
    nc.all_engine_barrier()
    assert self.sems is not None
    popped = nc._tile_sem_poison_stack.pop()
    assert popped is self._sem_poison
    nc.clear_and_free_semaphores(list(self.sems.allocated().values()))
    nc.all_engine_barrier()


tile.TileContext._drain_and_barrier = _drain_and_barrier


def _split_waits(nc):
    """walrus codegen accepts one sync-wait command per instruction; Tile can
    emit several. Hoist extras onto preceding same-engine NoOps."""
    for bb in nc.main_func.blocks:
        out = []
        for ins in bb.instructions:
            si = ins.sync_info
            waits = list(si.on_wait) if si and si.on_wait else []
            if len(waits) > 1:
                for w in waits[:-1]:
                    nop = mybir.InstNoOp(
                        name=nc.get_next_instruction_name(), ins=[], outs=[]
                    )
                    nop.engine = ins.engine
                    nop.sync_info = mybir.SyncInfo(on_wait=[w], on_update=[])
                    nc.register_instruction(nop)
                    out.append(nop)
                si.on_wait = [waits[-1]]
            out.append(ins)
        bb.instructions = out


# ---------------------------------------------------------------------------
# Model dimensions (hardcoded per problem spec)
# ---------------------------------------------------------------------------
N = 50000
E = 800000
G = 128
H = 8
C = 32
HC = H * C  # 256
P = 128
NCORES = 8
SLOTS = 127  # real slots per block (slot 127 = garbage)
MAXI16 = 25600  # table-piece size for int16 gather indices
NPC = N // NCORES  # 6250
NBLK = (NPC + SLOTS - 1) // SLOTS  # 50
CAP = NBLK * P  # 6400
CAPEXT = NCORES * CAP  # 51200
CH = 5  # gather tiles per dma_gather call
NQ = 4  # SWDGE queues to rotate gathers over


class Cfg:
    def __init__(self, tba, tbb):
        self.tba = tba
        self.tbb = tbb
        self.tb = tba + tbb
        self.ttot = NBLK * self.tb


# ---------------------------------------------------------------------------
# Host-side preprocessing
# ---------------------------------------------------------------------------


def _f32(a):
    return np.ascontiguousarray(a, dtype=np.float32)


def _bf(a):
    return np.ascontiguousarray(np.asarray(a, dtype=np.float32).astype(BF16))


def _wrap_idx(ids):
    """Gather-index list -> [128, len/16] int16 in the SWDGE wrap layout
    (idx j read from [j % 16, j // 16], replicated over the 8 Q7 cores)."""
    ids = np.asarray(ids, np.int16)
    assert len(ids) % 16 == 0
    w = ids.reshape(-1, 16).T  # [16, s]
    return np.tile(w, (8, 1))  # [128, s]


def _plan_blocks(edge_index):
    """LPT-pack dst nodes into blocks; return assignment + per-core edge
    structures + uniform tile counts."""
    src = np.concatenate([edge_index[0], np.arange(N)]).astype(np.int64)
    dst = np.concatenate([edge_index[1], np.arange(N)]).astype(np.int64)
    pieceB = (src // NPC) >= (NCORES // 2)
    dega = np.bincount(dst[~pieceB], minlength=N)
    degb = np.bincount(dst[pieceB], minlength=N)

    blk_of = np.empty(N, np.int64)
    slot_of = np.empty(N, np.int64)
    for c in range(NCORES):
        lo = c * NPC
        da = dega[lo : lo + NPC].astype(np.float64)
        db = degb[lo : lo + NPC].astype(np.float64)
        order = np.argsort(-(da + db), kind="stable")
        blk_a = np.zeros(NBLK)
        blk_b = np.zeros(NBLK)
        blk_n = np.zeros(NBLK, np.int64)
        for i in order:
            cost = np.maximum(blk_a + da[i], blk_b + db[i])
            cost[blk_n >= SLOTS] = np.inf
            j = int(np.argmin(cost))
            blk_of[lo + i] = j
            slot_of[lo + i] = blk_n[j]
            blk_a[j] += da[i]
            blk_b[j] += db[i]
            blk_n[j] += 1
    row_of = blk_of * P + slot_of  # within-core table row
    ext_row = (np.arange(N) // NPC) * CAP + row_of  # global table row

    # per-(core, block, piece) edge lists
    esrow = ext_row[src]
    eslot = slot_of[dst]
    eblk = blk_of[dst]
    ecore = dst // NPC
    percore = []
    na = np.zeros((NCORES, NBLK), int)
    nb_ = np.zeros((NCORES, NBLK), int)
    for c in range(NCORES):
        blocks = []
        selc = ecore == c
        for b in range(NBLK):
            sel = selc & (eblk == b)
            sa = sel & ~pieceB
            sb = sel & pieceB
            ra, la = esrow[sa], eslot[sa]
            rb, lb = esrow[sb] - MAXI16, eslot[sb]
            blocks.append((ra, la, rb, lb))
            na[c, b] = len(ra)
            nb_[c, b] = len(rb)
        percore.append(blocks)
    tba = int(math.ceil(na.max() / P))
    tbb = int(math.ceil(nb_.max() / P))
    return percore, row_of, Cfg(tba, tbb)


def _prep(x, batch, u, weights, cfg: Cfg, percore, row_of):
    att1 = weights["att1"]
    att2 = weights["att2"]

    def att_rep(att):
        return _bf(np.broadcast_to(att.reshape(-1), (P, HC)))

    tba, tbb, tb = cfg.tba, cfg.tbb, cfg.tb
    iota = np.arange(P)

    maps = []
    for c in range(NCORES):
        m = {}
        lo = c * NPC
        ixa = np.zeros((P, NBLK * tba * 8), np.int16)
        ixb = np.zeros((P, NBLK * tbb * 8), np.int16)
        Ot = np.zeros((P, NBLK * tb * P), BF16)
        OtT = np.zeros((P, NBLK * tb * P), BF16)
        for b in range(NBLK):
            ra, la, rb, lb = percore[c][b]
            ia = np.zeros(tba * P, np.int64)
            ia[: len(ra)] = ra
            ib = np.zeros(tbb * P, np.int64)
            ib[: len(rb)] = rb
            ixa[:, b * tba * 8 : (b + 1) * tba * 8] = _wrap_idx(ia)
            ixb[:, b * tbb * 8 : (b + 1) * tbb * 8] = _wrap_idx(ib)
            slots = np.full(tb * P, 127, np.int64)
            slots[: len(la)] = la
            slots[tba * P : tba * P + len(lb)] = lb
            oh = (slots[:, None] == iota[None, :]).astype(BF16)  # [tb*P, P]
            oh = oh.reshape(tb, P, P)
            cols = slice(b * tb * P, (b + 1) * tb * P)
            Ot[:, cols] = oh.transpose(1, 0, 2).reshape(P, tb * P)
            OtT[:, cols] = oh.transpose(2, 0, 1).reshape(P, tb * P)
        m["ixa"] = ixa
        m["ixb"] = ixb
        m["Ot"] = np.ascontiguousarray(Ot)
        m["OtT"] = np.ascontiguousarray(OtT)

        rows = row_of[lo : lo + NPC]
        xs = np.zeros((CAP, x.shape[1]), np.float32)
        xs[rows] = x[lo : lo + NPC]
        m["xT"] = _bf(xs.T)

        gsel = np.zeros((CAP, G), np.float32)
        gsel[rows, np.asarray(batch[lo : lo + NPC])] = 1.0
        m["gsel"] = _bf(gsel)
        maps.append(m)

    counts = np.bincount(np.asarray(batch), minlength=G).astype(np.float32)
    shared = {
        "Wl1": _bf(weights["Wl1"]),
        "Wr1": _bf(weights["Wr1"]),
        "Wl2": _bf(weights["Wl2"]),
        "Wr2": _bf(weights["Wr2"]),
        "att1r": att_rep(att1),
        "att2r": att_rep(att2),
        "b1r": _bf(np.broadcast_to(weights["b1"], (P, HC))),
        "b2r": _bf(np.broadcast_to(weights["b2"], (P, HC))),
        "ident": _bf(np.eye(P, dtype=np.float32)),
        "crecip": _f32((1.0 / np.maximum(counts, 1.0)).reshape(G, 1)),
        "Wlin1": _bf(weights["W_lin1"]),
        "blin1r": _f32(np.broadcast_to(weights["b_lin1"], (G, 64))),
        "Wout": _bf(weights["W_out"]),
        "boutr": _f32(np.full((G, 1), float(weights["b_out"][0]), np.float32)),
        "ub": _bf(u),
    }
    for m in maps:
        m.update(shared)
    return maps


# ---------------------------------------------------------------------------
# Device program
# ---------------------------------------------------------------------------


def _bcast_mid(ap, reps):
    return ap.unsqueeze(1).broadcast_to([ap.shape[0], reps, ap.shape[1]])


def _build(cfg: Cfg, in_dim=3):
    dt = mybir.dt
    bf = dt.bfloat16
    f32 = dt.float32
    nc = bacc.Bacc(None, num_swdge_queues=NQ) if NQ > 1 else bacc.Bacc(None)
    groups = [list(range(NCORES))]
    tba, tbb, tb = cfg.tba, cfg.tbb, cfg.tb

    def prm(name, shape, dtype):
        return nc.declare_dram_parameter(name, list(shape), dtype, isOutput=False)

    xT = prm("xT", [in_dim, CAP], bf)
    ixa = prm("ixa", [P, NBLK * tba * 8], dt.int16)
    ixb = prm("ixb", [P, NBLK * tbb * 8], dt.int16)
    Otp = prm("Ot", [P, NBLK * tb * P], bf)
    OtTp = prm("OtT", [P, NBLK * tb * P], bf)
    Wl1p = prm("Wl1", [in_dim, HC], bf)
    Wr1p = prm("Wr1", [in_dim, HC], bf)
    Wl2p = prm("Wl2", [HC, HC], bf)
    Wr2p = prm("Wr2", [HC, HC], bf)
    att1r = prm("att1r", [P, HC], bf)
    att2r = prm("att2r", [P, HC], bf)
    b1r = prm("b1r", [P, HC], bf)
    b2r = prm("b2r", [P, HC], bf)
    identp = prm("ident", [P, P], bf)
    gselp = prm("gsel", [CAP, G], bf)
    crecip = prm("crecip", [G, 1], f32)
    Wlin1 = prm("Wlin1", [HC, 64], bf)
    blin1r = prm("blin1r", [G, 64], f32)
    Woutp = prm("Wout", [64 + 3, 1], bf)
    boutr = prm("boutr", [G, 1], f32)
    ub = prm("ub", [G, 3], bf)
    out_g = nc.declare_dram_parameter("out_g", [G, 1], f32, isOutput=True)

    with tile.TileContext(nc) as tc:
        with (
            tc.tile_pool(name="const", bufs=1) as constp,
            tc.tile_pool(name="meta", bufs=3) as metap,
            tc.tile_pool(name="gbuf", bufs=2) as gbufp,
            tc.tile_pool(name="work", bufs=2) as workp,
            tc.tile_pool(name="small", bufs=3) as smallp,
            tc.tile_pool(name="psU", bufs=2, space="PSUM") as psU,
            tc.tile_pool(name="psA", bufs=1, space="PSUM") as psA,
            tc.tile_pool(name="psB", bufs=2, space="PSUM") as psB,
            tc.tile_pool(name="psG", bufs=1, space="PSUM") as psG,
            tc.tile_pool(name="dram", bufs=1, space="DRAM") as dram,
        ):
            # ---- constants to SBUF ----
            def cload(p):
                t = constp.tile([p.shape[0], p.shape[1]], p.dtype, name=p.name + "_s")
                nc.sync.dma_start(out=t[:], in_=p[:])
                return t

            def cload_k(p):
                nk = (p.shape[0] + P - 1) // P
                out = []
                for kt in range(nk):
                    rows = slice(kt * P, min((kt + 1) * P, p.shape[0]))
                    t = constp.tile(
                        [rows.stop - rows.start, p.shape[1]], p.dtype,
                        name=f"{p.name}_s{kt}",
                    )
                    nc.sync.dma_start(out=t[:], in_=p[rows, :])
                    out.append(t)
                return out

            xT_s = cload(xT)
            Wl1_s = cload_k(Wl1p)
            Wr1_s = cload_k(Wr1p)
            Wl2_s = cload_k(Wl2p)
            Wr2_s = cload_k(Wr2p)
            att1r_s = cload(att1r)
            att2r_s = cload(att2r)
            b1r_s = cload(b1r)
            b2r_s = cload(b2r)
            ident_s = cload(identp)
            crecip_s = cload(crecip)
            Wlin1_s = cload_k(Wlin1)
            blin1r_s = cload(blin1r)
            Wout_s = cload(Woutp)
            boutr_s = cload(boutr)
            ub_s = cload(ub)

            # ---- internal DRAM ----
            xl1_own = dram.tile([CAP, HC], bf)
            xr1_tab = dram.tile([CAP, HC], bf)
            xl1_ext = dram.tile([CAPEXT, HC], bf, addr_space="Shared")
            xl2_own = dram.tile([CAP, HC], bf)
            xr2_tab = dram.tile([CAP, HC], bf)
            xl2_ext = dram.tile([CAPEXT, HC], bf, addr_space="Shared")
            gp_in = dram.tile([G, HC], f32)
            gp_out = dram.tile([G, HC], f32, addr_space="Shared")

            A_ = mybir.AluOpType
            AF = mybir.ActivationFunctionType

            # ================= node tables =================
            def node_tables(lhsT_tiles, W_s, tab):
                for b in range(NBLK):
                    ps = psA.tile([P, HC], f32, tag="a")
                    lts = lhsT_tiles(b)
                    assert len(lts) == len(W_s)
                    for i, lt in enumerate(lts):
                        nc.tensor.matmul(
                            ps[:], lhsT=lt, rhs=W_s[i][:],
                            start=(i == 0), stop=(i == len(lts) - 1),
                        )
                    ev = smallp.tile([P, HC], bf, tag="tabev")
                    nc.scalar.activation(out=ev[:], in_=ps[:], func=AF.Copy)
                    nc.sync.dma_start(out=tab[b * P : (b + 1) * P, :], in_=ev[:])

            def x_lhsT(b):
                return [xT_s[:, b * P : (b + 1) * P]]

            # ================= edge pipeline =================
            qctr = [0]

            def edge_layer(xl_ext, xr_tab, attr_s, br_s, layer):
                gpool_ps = None
                if layer == 2:
                    gpool_ps = psG.tile([G, HC], f32, name=f"gpool_ps{layer}")

                def front(b):
                    """Block prologue: meta loads, gathers, one-hot xr expand.
                    Emitted one block ahead so these fill other blocks' stalls."""
                    st = {}
                    cols = slice(b * tb * P, (b + 1) * tb * P)
                    Ot_blk = metap.tile([P, tb * P], bf, tag="Ot")
                    nc.sync.dma_start(out=Ot_blk[:], in_=Otp[:, cols])
                    OtT_blk = metap.tile([P, tb * P], bf, tag="OtT")
                    nc.sync.dma_start(out=OtT_blk[:], in_=OtTp[:, cols])
                    ixa_t = metap.tile([P, tba * 8], dt.int16, tag="ixa")
                    nc.sync.dma_start(
                        out=ixa_t[:], in_=ixa[:, b * tba * 8 : (b + 1) * tba * 8]
                    )
                    ixb_t = metap.tile([P, tbb * 8], dt.int16, tag="ixb")
                    nc.sync.dma_start(
                        out=ixb_t[:], in_=ixb[:, b * tbb * 8 : (b + 1) * tbb * 8]
                    )
                    xr_blk = metap.tile([P, HC], bf, tag="xr")
                    nc.sync.dma_start(out=xr_blk[:], in_=xr_tab[b * P : (b + 1) * P, :])
                    if layer == 2:
                        gsel_blk = metap.tile([P, G], bf, tag="gselb")
                        nc.sync.dma_start(
                            out=gsel_blk[:], in_=gselp[b * P : (b + 1) * P, :]
                        )
                        st["gsel"] = gsel_blk
                    xrb_blk = smallp.tile([P, HC], bf, tag="xrb")
                    nc.vector.tensor_tensor(
                        out=xrb_blk[:], in0=br_s[:], in1=xr_blk[:], op=A_.subtract
                    )

                    gxl = gbufp.tile([P, tb, HC], bf, tag="gxl")

                    def chunked_gather(dst_t0, n_tiles, table, idxt):
                        for q0 in range(0, n_tiles, CH):
                            q1 = min(q0 + CH, n_tiles)
                            nc.gpsimd.dma_gather(
                                out_ap=gxl[:, dst_t0 + q0 : dst_t0 + q1, :],
                                in_ap=table,
                                idxs_ap=idxt[:, q0 * 8 : q1 * 8],
                                num_idxs=(q1 - q0) * P, num_idxs_reg=(q1 - q0) * P,
                                elem_size=HC,
                                queue_num=qctr[0] % NQ,
                            )
                            qctr[0] += 1

                    chunked_gather(0, tba, xl_ext[0:MAXI16, :], ixa_t)
                    chunked_gather(tba, tbb, xl_ext[MAXI16:CAPEXT, :], ixb_t)

                    xre = gbufp.tile([P, tb, HC], bf, tag="xre")
                    for t0 in range(0, tb, 4):
                        k = min(4, tb - t0)
                        ps = psU.tile([P, 4, HC], f32, tag="u")
                        for u_ in range(k):
                            t_ = t0 + u_
                            nc.tensor.matmul(
                                ps[:, u_, :],
                                lhsT=OtT_blk[:, t_ * P : (t_ + 1) * P],
                                rhs=xr_blk[:], start=True, stop=True,
                            )
                        nc.scalar.activation(
                            out=xre[:, t0 : t0 + k, :], in_=ps[:, 0:k, :],
                            func=AF.Copy,
                        )
                    st.update(Ot=Ot_blk, gxl=gxl, xre=xre, xrb=xrb_blk)
                    return st

                def back(b, st):
                    Ot_blk, gxl, xre, xrb_blk = st["Ot"], st["gxl"], st["xre"], st["xrb"]
                    ut = workp.tile([P, tb, HC], bf, tag="ut")
                    nc.vector.tensor_tensor(out=ut[:], in0=gxl[:], in1=xre[:], op=A_.add)
                    ft = workp.tile([P, tb, HC], bf, tag="ft")
                    nc.scalar.activation(out=ft[:], in_=ut[:], func=AF.Prelu, alpha=0.2)
                    Pt = workp.tile([P, tb, HC], bf, tag="Pt")
                    nc.vector.tensor_tensor(
                        out=Pt[:], in0=ft[:], in1=_bcast_mid(attr_s[:], tb), op=A_.mult
                    )
                    v = Pt[:].rearrange("p t (h c) -> p (t h) c", h=H)
                    t1 = workp.tile([P, tb * H, 16], bf, tag="t1")
                    nc.vector.tensor_tensor(out=t1[:], in0=v[:, :, 0:16], in1=v[:, :, 16:32], op=A_.add)
                    t2 = workp.tile([P, tb * H, 8], bf, tag="t2")
                    nc.vector.tensor_tensor(out=t2[:], in0=t1[:, :, 0:8], in1=t1[:, :, 8:16], op=A_.add)
                    t3 = workp.tile([P, tb * H, 4], bf, tag="t3")
                    nc.vector.tensor_tensor(out=t3[:], in0=t2[:, :, 0:4], in1=t2[:, :, 4:8], op=A_.add)
                    t4 = workp.tile([P, tb * H, 2], bf, tag="t4")
                    nc.vector.tensor_tensor(out=t4[:], in0=t3[:, :, 0:2], in1=t3[:, :, 2:4], op=A_.add)
                    lg2 = workp.tile([P, tb * H, 2], bf, tag="lg2")
                    nc.vector.tensor_tensor(
                        out=lg2[:],
                        in0=t4[:, :, 0:1].broadcast_to([P, tb * H, 2]),
                        in1=t4[:, :, 1:2].broadcast_to([P, tb * H, 2]),
                        op=A_.add,
                    )
                    ex2 = workp.tile([P, tb * H, 2], bf, tag="ex2")
                    nc.scalar.activation(out=ex2[:], in_=lg2[:], func=AF.Exp)
                    exv = ex2[:].rearrange("p (t h) j -> p t h j", t=tb)
                    msg = workp.tile([P, tb, HC], bf, tag="msg")
                    nc.vector.tensor_tensor(
                        out=msg[:].rearrange("p t (h k j) -> p (t h) k j", h=H, j=2),
                        in0=ut[:].rearrange("p t (h k j) -> p (t h) k j", h=H, j=2),
                        in1=ex2[:].unsqueeze(2).broadcast_to([P, tb * H, 16, 2]),
                        op=A_.mult,
                    )

                    psacc = psB.tile([P, HC + H], f32, tag="b")
                    acc = psacc[:, 0:HC]
                    accd = psacc[:, HC : HC + H]
                    for t in range(tb):
                        Ot_t = Ot_blk[:, t * P : (t + 1) * P]
                        nc.tensor.matmul(
                            acc, lhsT=Ot_t, rhs=msg[:, t, :],
                            start=(t == 0), stop=(t == tb - 1),
                        )
                        # start=False even at t==0: acc's start=True already
                        # cleared the whole bank's has_written bits, so the
                        # first accd matmul overwrites (bit unset) rather than
                        # accumulating onto garbage; a second start=True here
                        # would re-clear the bank and drop acc's tile-0 sums.
                        nc.tensor.matmul(
                            accd, lhsT=Ot_t, rhs=exv[:, t, :, 0],
                            start=False, stop=(t == tb - 1),
                        )

                    denom = smallp.tile([P, H], f32, tag="denom")
                    nc.vector.tensor_scalar(
                        out=denom[:], in0=accd, scalar1=1e-20, scalar2=None,
                        op0=A_.max,
                    )
                    rec = smallp.tile([P, H], f32, tag="rec")
                    nc.vector.reciprocal(out=rec[:], in_=denom[:])
                    hsc = smallp.tile([P, HC], bf, tag="hsc")
                    nc.vector.tensor_tensor(
                        out=hsc[:].rearrange("p (h c) -> p h c", h=H),
                        in0=acc.rearrange("p (h c) -> p h c", h=H),
                        in1=rec[:].to_broadcast([P, H, C]),
                        op=A_.mult,
                    )
                    hfin = smallp.tile([P, HC], bf, tag="hfin")
                    nc.vector.tensor_tensor(out=hfin[:], in0=hsc[:], in1=xrb_blk[:], op=A_.add)
                    hout = smallp.tile([P, HC], bf, tag="hout")
                    nc.vector.tensor_scalar(
                        out=hout[:], in0=hfin[:], scalar1=0.0, scalar2=None,
                        op0=A_.max,
                    )

                    if layer == 1:
                        # transpose h1 block and compute layer-2 node tables
                        # inline (h1T never round-trips through DRAM).
                        tps = []
                        for kt in range(2):
                            tp = psA.tile([P, P], bf, tag="a")
                            nc.tensor.transpose(
                                out=tp[:], in_=hout[:, kt * P : (kt + 1) * P],
                                identity=ident_s[:],
                            )
                            t_ = smallp.tile([P, P], bf, tag="htps")
                            nc.vector.tensor_copy(out=t_[:], in_=tp[:])
                            tps.append(t_)
                        for W_s, tab in ((Wl2_s, xl2_own), (Wr2_s, xr2_tab)):
                            ps2 = psA.tile([P, HC], f32, tag="a")
                            for i in range(2):
                                nc.tensor.matmul(
                                    ps2[:], lhsT=tps[i][:], rhs=W_s[i][:],
                                    start=(i == 0), stop=(i == 1),
                                )
                            ev = smallp.tile([P, HC], bf, tag="tabev")
                            nc.vector.tensor_copy(out=ev[:], in_=ps2[:])
                            nc.sync.dma_start(
                                out=tab[b * P : (b + 1) * P, :], in_=ev[:]
                            )
                    else:
                        nc.tensor.matmul(
                            gpool_ps[:], lhsT=st["gsel"][:], rhs=hout[:],
                            start=(b == 0), stop=(b == NBLK - 1),
                        )

                st = front(0)
                for b in range(NBLK):
                    nxt = front(b + 1) if b + 1 < NBLK else None
                    back(b, st)
                    st = nxt
                return gpool_ps

            # ================= layer 1 =================
            node_tables(x_lhsT, Wl1_s, xl1_own)
            nc.gpsimd.collective_compute(
                "AllGather", A_.bypass, replica_groups=groups,
                ins=[xl1_own.opt()], outs=[xl1_ext.opt()],
            )
            node_tables(x_lhsT, Wr1_s, xr1_tab)
            edge_layer(xl1_ext, xr1_tab, att1r_s, b1r_s, layer=1)

            # ================= layer 2 =================
            nc.gpsimd.collective_compute(
                "AllGather", A_.bypass, replica_groups=groups,
                ins=[xl2_own.opt()], outs=[xl2_ext.opt()],
            )
            gpool_ps = edge_layer(xl2_ext, xr2_tab, att2r_s, b2r_s, layer=2)

            # ================= pool + MLP =================
            gsum = smallp.tile([G, HC], f32, tag="gsum")
            nc.scalar.activation(out=gsum[:], in_=gpool_ps[:], func=AF.Copy)
            nc.sync.dma_start(out=gp_in[:], in_=gsum[:])
            nc.gpsimd.collective_compute(
                "AllReduce", A_.add, replica_groups=groups,
                ins=[gp_in.opt()], outs=[gp_out.opt()],
            )
            gsum2 = smallp.tile([G, HC], f32, tag="gsum2")
            nc.sync.dma_start(out=gsum2[:], in_=gp_out[:])
            gmean = smallp.tile([G, HC], bf, tag="gmean")
            nc.vector.tensor_scalar(
                out=gmean[:], in0=gsum2[:], scalar1=crecip_s[:, 0:1], scalar2=None,
                op0=A_.mult,
            )
            gT = []
            for kt in range(2):
                tp = psA.tile([P, G], bf, tag="a")
                nc.tensor.transpose(
                    out=tp[:], in_=gmean[:, kt * P : (kt + 1) * P], identity=ident_s[:]
                )
                gkt = smallp.tile([P, G], bf, tag="gT", name=f"gT{kt}")
                nc.scalar.activation(out=gkt[:], in_=tp[:], func=AF.Copy)
                gT.append(gkt)
            lin_ps = psB.tile([G, 64], f32, tag="b")
            for kt in range(2):
                nc.tensor.matmul(
                    lin_ps[:], lhsT=gT[kt][:], rhs=Wlin1_s[kt][:],
                    start=(kt == 0), stop=(kt == 1),
                )
            lin = smallp.tile([G, 64], f32, tag="lin")
            nc.vector.tensor_tensor(out=lin[:], in0=lin_ps[:], in1=blin1r_s[:], op=A_.add)
            glu = smallp.tile([G, P], bf, tag="glu")
            nc.scalar.activation(out=glu[:, 0:64], in_=lin[:], func=AF.Relu)
            nc.vector.tensor_copy(out=glu[:, 64:67], in_=ub_s[:])
            nc.gpsimd.memset(glu[:, 67:P], 0.0)
            tp = psA.tile([P, G], bf, tag="a")
            nc.tensor.transpose(out=tp[:], in_=glu[:], identity=ident_s[:])
            gluT = smallp.tile([P, G], bf, tag="gluT")
            nc.scalar.activation(out=gluT[:], in_=tp[:], func=AF.Copy)
            out_ps = psB.tile([G, 1], f32, tag="b")
            nc.tensor.matmul(
                out_ps[:], lhsT=gluT[0:67, :], rhs=Wout_s[:], start=True, stop=True
            )
            outs = smallp.tile([G, 1], f32, tag="outs")
            nc.vector.tensor_tensor(out=outs[:], in0=out_ps[:], in1=boutr_s[:], op=A_.add)
            nc.sync.dma_start(out=out_g[:], in_=outs[:])

    nc.compile()
    _split_waits(nc)
    return nc


# ---------------------------------------------------------------------------
# Entry point
# ---------------------------------------------------------------------------


def kernel(**inputs):
    import os

    from concourse.bass_utils import run_bass_kernel_spmd

    x = np.asarray(inputs["x"], np.float32)
    edge_index = np.asarray(inputs["edge_index"], np.int64)
    batch = np.asarray(inputs["batch"], np.int64)
    u = np.asarray(inputs["u"], np.float32)
    weights = {
        k: np.asarray(inputs[k], np.float32)
        for k in ("Wl1", "Wr1", "att1", "b1", "Wl2", "Wr2", "att2", "b2",
                  "W_lin1", "b_lin1", "W_out", "b_out")
    }
    percore, row_of, cfg = _plan_blocks(edge_index)
    maps = _prep(x, batch, u, weights, cfg, percore, row_of)
    nc = _build(cfg, in_dim=x.shape[1])
    trace = bool(os.environ.get("KERNEL_TRACE"))
    try:
        res = run_bass_kernel_spmd(nc, maps, list(range(NCORES)), trace=trace)
    except ModuleNotFoundError:
        res = run_bass_kernel_spmd(nc, maps, list(range(NCORES)))
    if trace and getattr(res, "exec_time_ns", None) is not None:
        print(f"HW exec time: {res.exec_time_ns} ns")
    return res.results[0]["out_g"].reshape(G).astype(np.float32)
